# revision 26
# baseline (speedup 1.0000x reference)
"""Trainium2 Bass kernel for nn_DecoderBlockWithKeywords.

Decoder block: causal self-attn + gated (source-code / keywords) cross-attn
+ template cross-attn + FFN, with 4 LayerNorms.  B=4, T=1024, D=512, H=8,
dh=64, DFF=2048.

Sharding: pure data-parallel over (batch, query-half) -> 8 NeuronCores, no
collectives.  Each core holds all weights (fp16) and computes 512 query
tokens of one batch element.

Layout strategy: every activation lives feature-major (X^T: [D on
partitions, tokens on free]).  Host pre-transposes/casts inputs.  Q/K
projections are weight-stationary (out feature-major); V is produced
token-major via activation-stationary matmuls so the attention AV matmul
needs no transposes at all.  Scores are computed as S^T = K_h Q_h^T
([kv, q]); softmax runs without max-subtraction (logits are O(1); masked
lanes get -1e6 bias fused into the ACT exp).  Softmax denominators come
from a ones-column appended to V inside the same AV matmul; per-column
scales (softmax 1/n, LN mean/rstd, gate g) are broadcast across partitions
with a PE ones-outer-product into a free PSUM bank and applied by one DVE
op reading that PSUM operand.  LayerNorm is done feature-major: column sums
via PE ones-matmuls, rstd = exp(-0.5*ln(v)) on ACT (single activation-table
set, zero table switches).  Residuals follow the reference post-LN chaining
(z = LN(y + y2), z_end = LN(z + z2), out = LN(z_end + ff)).

Programs are specialized at build time to the actual kv lengths (read from
the int32 length inputs), so masked kv tiles are skipped entirely; up to 8
distinct programs (4 batches x even/odd query half) are compiled and
launched concurrently on disjoint device subsets.
"""

import os
import sys
import threading

import numpy as np

for _p in ("/opt/trn_rl_repo", "/root/.axon_site"):
    if os.path.isdir(_p) and _p not in sys.path:
        sys.path.append(_p)

import ml_dtypes
from contextlib import ExitStack

import concourse.bass as bass
import concourse.mybir as mybir
from concourse import bacc
from concourse.tile import TileContext

BF16 = np.float16
F32 = np.float32
F8 = ml_dtypes.float8_e4m3
WSCALE = 16.0      # weights stored as w*16 in fp8 (values ~N(0,0.02))
WINV = 1.0 / WSCALE
GSCALE = 64.0      # gate u-vectors stored as u*64 in fp8
NEG = -1000000.0
B, T, S, TM, KW, D, H, DFF = 4, 1024, 1024, 512, 64, 512, 8, 2048
DH = D // H  # 64
P = 128
NCH = D // P  # 4 feature chunks
AF = mybir.ActivationFunctionType
OP = mybir.AluOpType


# ---------------------------------------------------------------------------
# program builder
# ---------------------------------------------------------------------------

def build_program(qh, kts_cc, kts_ct, gate_b=0.0, apply_affine=False, debug=False):
    """Build one core's Bass program.

    qh: 0/1 query half.  kts_cc/kts_ct: number of 128-wide kv tiles for the
    source-code / template cross attentions (specialized to actual length).
    """
    f32, bf16 = mybir.dt.float32, mybir.dt.float16
    f8 = mybir.dt.float8e4
    DR = mybir.MatmulPerfMode.DoubleRow
    KV = 512 * (qh + 1)          # self-attn kv range
    QOFF = qh * 512              # q columns inside xkvT

    nc = bacc.Bacc("TRN2", target_bir_lowering=False, debug=False)

    def din(name, shape, dt=bf16):
        return nc.dram_tensor(name, shape, dt, kind="ExternalInput").ap()

    xkvT = din("xkvT", [D, KV], f8)
    xqT = din("xqT", [D, 512])       # bf16 x for the residual add
    srcT = din("srcT", [D, kts_cc * P], f8)
    tmplT = din("tmplT", [D, kts_ct * P], f8)
    kwT = din("kwT", [D, KW], f8)
    wnames = [f"{n}_{p}" for n in ("sa", "cc", "ct", "ck")
              for p in ("wq", "wk", "wv", "wo")]
    wd = {n: din(n, [D, D], f8) for n in wnames}
    w1bd = din("ffn_w1b", [D, DFF])
    w2bd = din("ffn_w2b", [DFF, D])
    gwA = din("gwA", [D, 1], f8)
    gwB = din("gwB", [D, 1], f8)
    staird = din("stair", [P, P])
    sel8_d = din("sel8", [H, NCH * P])
    ccbias_d = din("cc_bias", [P, 1], f32)
    ctbias_d = din("ct_bias", [P, 1], f32)
    kwbias_d = din("kw_bias", [KW, 1], f32)
    affine_d = din("ln_affine", [P, NCH * 8], f32) if apply_affine else None
    outT = nc.dram_tensor("outT", [D, 512], f32, kind="ExternalOutput").ap()
    dbg_outs = {}

    def mkdbg(nm, shape):
        if nm not in dbg_outs:
            dbg_outs[nm] = nc.dram_tensor(f"dbg_{nm}", shape, f32,
                                          kind="ExternalOutput").ap()
        return dbg_outs[nm]

    with TileContext(nc, pool_alloc_mode="queue") as tc, ExitStack() as ctx:
        # Pre-place one ACT table covering Exp+Ln+Copy+Square+Relu: without
        # this the compiler's greedy chooser ping-pongs exp_and_others <->
        # natural_log (~19 loads x 1.3us on the critical path).
        from concourse.hw_specs import get_activation_tables
        _tabs = list(get_activation_tables(nc.m.arch).keys())
        nc.scalar.add_instruction(mybir.InstLoadActFuncSet(
            act_func_set_id=_tabs.index("natural_log_exp_and_others"),
            name=nc.get_next_instruction_name(),
            engine=mybir.EngineType.Activation))
        pers = ctx.enter_context(tc.tile_pool(name="pers", bufs=1))
        # ---- persistent small constants -------------------------------
        # (const DMAs are emitted by emit_consts() AFTER the stage-1
        # critical weight/activation loads so they don't delay the first
        # projection)
        stair = pers.tile([P, P], bf16, name="stair_t")
        ccbias = pers.tile([P, 1], f32, name="ccbias_t")
        ctbias = pers.tile([P, 1], f32, name="ctbias_t")
        kwbias = pers.tile([KW, 1], f32, name="kwbias_t")
        gwa_t = pers.tile([P, NCH], f8, name="gwa_t")
        gwb_t = pers.tile([P, NCH], f8, name="gwb_t")
        sel8 = pers.tile([H, NCH * P], bf16, name="sel8_t")
        affine = None
        if apply_affine:
            affine = pers.tile([P, NCH * 8], f32, name="affine_t")

        def emit_consts():
            nc.sync.dma_start(out=stair, in_=staird)
            nc.sync.dma_start(out=ccbias, in_=ccbias_d)
            nc.sync.dma_start(out=ctbias, in_=ctbias_d)
            nc.sync.dma_start(out=kwbias, in_=kwbias_d)
            nc.sync.dma_start(out=gwa_t,
                              in_=gwA.rearrange("(i p) o -> p i o", p=P))
            nc.sync.dma_start(out=gwb_t,
                              in_=gwB.rearrange("(i p) o -> p i o", p=P))
            nc.sync.dma_start(out=sel8, in_=sel8_d)
            if apply_affine:
                nc.sync.dma_start(out=affine, in_=affine_d)

        ones_f = pers.tile([P, 1], f32, name="ones_f")
        nc.vector.memset(ones_f, 1.0)
        ones_b = pers.tile([P, 1], bf16, name="ones_b")
        nc.vector.memset(ones_b, 1.0)
        ones_row = pers.tile([1, P], bf16, name="ones_row")
        nc.vector.memset(ones_row, 1.0)
        eps_t = pers.tile([1, 1], f32, name="eps_t")
        nc.vector.memset(eps_t, 1e-5)
        gb_t = pers.tile([1, 1], f32, name="gb_t")
        nc.vector.memset(gb_t, -float(gate_b))

        def tap(nm, tiles):
            if not debug:
                return
            cols = tiles[0].shape[-1]
            d = mkdbg(nm, [len(tiles) * P, cols])
            for i, t in enumerate(tiles):
                rows = t.shape[0]
                nc.gpsimd.dma_start(out=d[i * P:i * P + rows, :], in_=t)

        # ---- global shared pools --------------------------------------
        # residual/LN-out tiles, reused across stages via shared tags
        rpool = ctx.enter_context(tc.tile_pool(name="rpool", bufs=1))

        def mktiles(nm, cols=512, dt=f32, n=NCH, tagp=None):
            tagp = tagp or nm
            return [rpool.tile([P, cols], dt, name=f"{nm}{i}", tag=f"{tagp}{i}",
                               bufs=1) for i in range(n)]

        # small 1/8-partition tiles + broadcast tiles, shared by all stages
        smallp = ctx.enter_context(tc.tile_pool(name="smallp", bufs=1))
        # transient [128, *] tiles (exp outputs, LN scratch, gate scratch)
        trp = ctx.enter_context(tc.tile_pool(name="trp", bufs=1))
        # PSUM: pps = projection/V accumulators; x_ps = paired scores
        # (2 banks each); x_po = AV out + LN stats + gate
        psA = ctx.enter_context(tc.tile_pool(name="psA", bufs=2, space="PSUM"))
        psB = ctx.enter_context(tc.tile_pool(name="psB", bufs=2, space="PSUM"))

        def load_w(pool, names):
            # fp8 weights in DoubleRow-paired layout: free dims
            # (ipair, two, out-col)
            for n in names:
                wt[n] = pool.tile([P, NCH * D], f8, name=f"{n}_t",
                                  tag=f"{n}_t", bufs=1)
                nc.sync.dma_start(
                    out=wt[n].rearrange("p (ip two n) -> p ip two n",
                                        ip=2, two=2),
                    in_=wd[n].rearrange("(ip two p) n -> p ip two n",
                                        p=P, two=2))
        wt = {}

        def w_lhsT(n, ip, j):
            # [128, 2, 128] stationary pair for DoubleRow
            return wt[n].rearrange("p (ip two n) -> p ip two n",
                                   ip=2, two=2)[:, ip, :, j * P:(j + 1) * P]

        def w_rhs(n, ip, cols=D):
            # [128, 2, cols] moving pair for DoubleRow
            return wt[n].rearrange("p (ip two n) -> p ip two n",
                                   ip=2, two=2)[:, ip, :, 0:cols]

        def load_act(pool, nm, dram_ap, cols):
            # fp8 activations as 2 chunk-paired tiles [P, 2, cols]
            tiles = []
            for pi in range(2):
                t = pool.tile([P, 2 * cols], f8, name=f"{nm}{pi}",
                              tag=f"{nm}{pi}", bufs=1)
                nc.sync.dma_start(
                    out=t.rearrange("p (two n) -> p two n", two=2),
                    in_=dram_ap[pi * 2 * P:(pi + 1) * 2 * P, :].rearrange(
                        "(two p) n -> p two n", p=P))
                tiles.append(t)
            return tiles

        def pair_view(t):
            return t.rearrange("p (two n) -> p two n", two=2)

        # ----------------------------------------------------------------
        # helpers
        # ----------------------------------------------------------------
        def proj_fm(wn, rhs_pairs, ncols, out_tiles, evict, coff=0):
            # rhs_pairs: 2 chunk-paired fp8 tiles [P, 2, >=coff+ncols]
            ntt = (ncols + 511) // 512
            for j in range(NCH):
                for t in range(ntt):
                    cs = t * 512
                    ce = min(ncols, cs + 512)
                    ps = psA.tile([P, ce - cs], mybir.dt.float32,
                                  name="proj_ps", tag="pps")
                    for ip in range(2):
                        nc.tensor.matmul(
                            ps, w_lhsT(wn, ip, j),
                            pair_view(rhs_pairs[ip])[:, :,
                                                     coff + cs:coff + ce],
                            start=(ip == 0), stop=(ip == 1),
                            perf_mode=DR)
                    evict(j, cs, ce, ps, out_tiles)

        def evict_copy(j, cs, ce, ps, out_tiles):
            nc.vector.tensor_scalar_mul(out_tiles[j][:, cs:ce], ps, WINV)

        def proj_v(enc_pairs, wn, nkv, vt_list, vpool, ktag):
            # produces kv-tile-PAIRED fp8 V tiles [rows, nsub, H*(DH+1)]
            # (nsub=2 except a trailing odd tile) for DoubleRow AV matmuls
            nch_tok = (nkv + P - 1) // P
            for m in range(nch_tok):
                rows = min(P, nkv - m * P)
                ps = psA.tile([rows, D], mybir.dt.float32,
                              name="v_ps", tag="pps")
                for ip in range(2):
                    nc.tensor.matmul(
                        ps, pair_view(enc_pairs[ip])[:, :, m * P:m * P + rows],
                        w_rhs(wn, ip),
                        start=(ip == 0), stop=(ip == 1), perf_mode=DR)
                if m % 2 == 0:
                    nsub = 2 if m + 1 < nch_tok else 1
                    vtp = vpool.tile([rows, nsub * H * (DH + 2)], f8,
                                     name=f"{ktag}_v{m // 2}",
                                     tag=f"{ktag}_v{m // 2}", bufs=1)
                    vt_list.append((vtp, nsub, rows))
                vtp, nsub, _ = vt_list[-1]
                v4 = vtp.rearrange("p (two g c) -> p two g c",
                                   two=nsub, c=DH + 2)
                nc.any.tensor_scalar_mul(
                    v4[:, m % 2, :, 0:DH],
                    ps.rearrange("p (g c) -> p g c", c=DH), WINV)
                nc.vector.memset(v4[:, m % 2, :, DH:DH + 2], 1.0)

        def attention(qt, kt, vt_pairs, out_tiles, out8, bias_tile, causal,
                      ktag):
            """Multi-head attention, head pairs share one [rows,1024]
            scores psum + one merged exp (fp8 out, kv-pair planes).  AV is
            a DoubleRow fp8 matmul over kv-tile pairs.  Causal tiles only
            touch live query columns.  Returns a finish() closure that
            emits the normalizer broadcasts + final muls, so callers can
            interleave independent PE work with the 1/n scalar chain."""
            nkt = sum(ns for _, ns, _ in vt_pairs)
            npairs = len(vt_pairs)
            nmat = smallp.tile([H, 512], mybir.dt.float32,
                               name=f"{ktag}_nmat", tag="nmat", bufs=1)
            for hp in range(H // 2):
                po = []
                for s in range(2):
                    po.append(psB.tile([DH + 2, 512], mybir.dt.float32,
                                       name=f"{ktag}_po{s}", tag="x_po"))
                kt_base = 0
                for mp, (vtp, nsub, vrows) in enumerate(vt_pairs):
                    ds = [(kt_base + u - (nkt - 4)) if causal else -1
                          for u in range(nsub)]
                    c0s = [d * P if (causal and d > 0) else 0 for d in ds]
                    c0p = c0s[0]
                    pp = trp.tile([vrows, nsub * 1024], f8,
                                  name=f"{ktag}_pt", tag="pt", bufs=3)
                    pp3 = pp.rearrange("p (two n) -> p two n", two=nsub)
                    for u in range(nsub):
                        kt_i = kt_base + u
                        c0 = c0s[u]
                        d = ds[u]
                        ps2 = psB.tile([vrows, 1024], mybir.dt.float32,
                                       name=f"{ktag}_ps", tag="x_ps")
                        for s in range(2):
                            ro = s * DH
                            o = s * 512
                            nc.tensor.matmul(
                                ps2[:, o + c0:o + 512],
                                kt[hp][ro:ro + DH,
                                       kt_i * P:kt_i * P + vrows],
                                qt[hp][ro:ro + DH, c0:512],
                                start=True, stop=True)
                        if causal and d >= 0:
                            for s in range(2):
                                o = s * 512
                                nc.vector.tensor_add(
                                    ps2[:, o + c0:o + c0 + P],
                                    ps2[:, o + c0:o + c0 + P], stair)
                                if c0 > c0p:
                                    nc.vector.memset(
                                        pp3[:, u, o + c0p:o + c0], 0.0)
                            # one strided exp covers both heads' live range
                            nc.scalar.activation(
                                pp.rearrange(
                                    "p (two s n) -> p two s n",
                                    two=nsub, s=2)[:, u, :, c0:512],
                                ps2.rearrange("p (s n) -> p s n",
                                              s=2)[:, :, c0:512],
                                AF.Exp, scale=0.125)
                        else:
                            bias = 0.0
                            if bias_tile is not None and kt_i == nkt - 1:
                                bias = bias_tile[:vrows, :]
                            nc.scalar.activation(pp3[:, u, :], ps2, AF.Exp,
                                                 bias=bias, scale=0.125)
                    v4 = vtp.rearrange("p (two g c) -> p two g c",
                                       two=nsub, c=DH + 2)
                    for s in range(2):
                        h = 2 * hp + s
                        o = s * 512
                        if nsub == 2:
                            nc.tensor.matmul(
                                po[s][:, c0p:512], v4[:, :, h, :],
                                pp3[:, :, o + c0p:o + 512],
                                start=(mp == 0), stop=(mp == npairs - 1),
                                skip_group_check=True, perf_mode=DR)
                        else:
                            nc.tensor.matmul(
                                po[s][:, c0p:512], v4[:, 0, h, :],
                                pp3[:, 0, o + c0p:o + 512],
                                start=(mp == 0), stop=(mp == npairs - 1),
                                skip_group_check=True)
                    kt_base += nsub
                for s in range(2):
                    ro = s * DH
                    h = 2 * hp + s
                    nscr = smallp.tile([1, 512], mybir.dt.float32,
                                       name=f"{ktag}_nscr{h}", tag="nscr",
                                       bufs=2)
                    nc.any.tensor_copy(nscr, po[s][DH:DH + 1, :])
                    nc.sync.dma_start(out=nmat[h:h + 1, :], in_=nscr)
                    nc.any.tensor_copy(out_tiles[hp][ro:ro + DH, :],
                                       po[s][0:DH, :])
            lnn = smallp.tile([H, 512], mybir.dt.float32,
                              name=f"{ktag}_lnn", tag="lnn", bufs=1)
            nc.scalar.activation(lnn, nmat, AF.Ln)
            ninv8 = smallp.tile([H, 512], bf16,
                                name=f"{ktag}_ninv8", tag=f"ninv8_{ktag}",
                                bufs=1)
            nc.scalar.activation(ninv8, lnn, AF.Exp, scale=-1.0)

            def finish():
                for hp in range(H // 2):
                    nb = psA.tile([P, 512], mybir.dt.float32,
                                  name=f"{ktag}_nb", tag="pps")
                    nc.tensor.matmul(nb, sel8[:, hp * P:(hp + 1) * P],
                                     ninv8, start=True, stop=True)
                    nc.vector.tensor_mul(
                        pair_view(out8[hp // 2])[:, hp % 2, :],
                        out_tiles[hp], nb)
            return finish

        def layernorm(r_tiles, out_tiles, ln_idx, out8=None):
            sq = [trp.tile([P, 512], bf16, name=f"ln{ln_idx}_sq", tag="ln_sq",
                           bufs=4) for _ in range(NCH)]
            for j in range(NCH):
                nc.gpsimd.tensor_mul(sq[j], r_tiles[j], r_tiles[j])
            ps_s = psB.tile([1, 512], mybir.dt.float32,
                            name="ln_ps_s", tag="x_po")
            ps_q = psB.tile([1, 512], mybir.dt.float32,
                            name="ln_ps_q", tag="x_po")
            for j in range(NCH):
                nc.tensor.matmul(ps_s, ones_b, r_tiles[j],
                                 start=(j == 0), stop=(j == NCH - 1))
            for j in range(NCH):
                nc.tensor.matmul(ps_q, ones_b, sq[j],
                                 start=(j == 0), stop=(j == NCH - 1))
            mean16 = smallp.tile([1, 512], bf16,
                                 name="ln_mean16", tag="ln_stat", bufs=3)
            nc.vector.tensor_scalar_mul(mean16, ps_s, 1.0 / D)
            msq = smallp.tile([1, 512], mybir.dt.float32,
                              name="ln_msq", tag="ln_stat", bufs=3)
            nc.vector.tensor_mul(msq, mean16, mean16)
            var = smallp.tile([1, 512], mybir.dt.float32,
                              name="ln_var", tag="ln_stat", bufs=3)
            nc.vector.scalar_tensor_tensor(var, ps_q, 1.0 / D, msq,
                                           op0=OP.mult, op1=OP.subtract)
            lnv = smallp.tile([1, 512], mybir.dt.float32,
                              name="ln_lnv", tag="ln_stat", bufs=3)
            nc.scalar.activation(lnv, var, AF.Ln, bias=eps_t[:, :])
            rstd = smallp.tile([1, 512], bf16,
                               name="ln_rstd", tag="ln_stat", bufs=3)
            nc.scalar.activation(rstd, lnv, AF.Exp, scale=-0.5)

            def apply():
                meanb = psB.tile([P, 512], mybir.dt.float32,
                                 name="ln_meanb", tag="x_po")
                nc.tensor.matmul(meanb, ones_row, mean16,
                                 start=True, stop=True)
                rstdb = psB.tile([P, 512], mybir.dt.float32,
                                 name="ln_rstdb", tag="x_po")
                nc.tensor.matmul(rstdb, ones_row, rstd,
                                 start=True, stop=True)
                for j in range(NCH):
                    tmp = trp.tile([P, 512], bf16,
                                   name="ln_tmp", tag="ln_tmp", bufs=2)
                    nc.vector.tensor_sub(tmp, r_tiles[j], meanb)
                    nc.vector.tensor_mul(out_tiles[j], tmp, rstdb)
                    if apply_affine:
                        g = affine[:, ln_idx * 2 * NCH + j:
                                   ln_idx * 2 * NCH + j + 1]
                        b = affine[:, ln_idx * 2 * NCH + NCH + j:
                                   ln_idx * 2 * NCH + NCH + j + 1]
                        nc.vector.tensor_scalar(out_tiles[j], out_tiles[j],
                                                g, b, op0=OP.mult,
                                                op1=OP.add)
                    if out8 is not None:
                        nc.gpsimd.tensor_copy(
                            pair_view(out8[j // 2])[:, j % 2, :],
                            out_tiles[j])
            return apply

        # ================================================================
        # emission (ordered for cross-stage overlap)
        # ================================================================
        r1 = mktiles("r1", dt=bf16, tagp="rA")
        y = mktiles("y", dt=bf16, tagp="lnA")
        r2 = mktiles("r2", dt=bf16, tagp="rB")
        z = mktiles("z", dt=bf16, tagp="lnB")
        r3 = None  # allocated after r1 dies
        ze = None

        ccsb = ctx.enter_context(tc.tile_pool(name="cc_sb", bufs=1))
        sasb_cm = tc.tile_pool(name="sa_sb", bufs=1)
        sasb = sasb_cm.__enter__()

        def mk_at8(pool, nm):
            return [pool.tile([P, 1024], f8, name=f"{nm}{i}",
                              tag=f"at8_{i}", bufs=2) for i in range(2)]

        # --- stage 1: self attention ---
        load_w(sasb, ["sa_wk"])
        xkv = load_act(sasb, "xkv", xkvT, KV)
        xq = []
        for i in range(NCH):
            t = sasb.tile([P, 512], bf16, name=f"xq{i}", tag=f"xq{i}",
                          bufs=1)
            nc.sync.dma_start(out=t, in_=xqT[i * P:(i + 1) * P, :])
            xq.append(t)
        load_w(sasb, ["sa_wv", "sa_wq", "sa_wo"])
        emit_consts()
        qt = [sasb.tile([P, 512], bf16, name=f"sa_q{i}", tag=f"sa_q{i}",
                        bufs=1) for i in range(NCH)]
        ktl = [sasb.tile([P, KV], bf16, name=f"sa_k{i}", tag=f"sa_k{i}",
                         bufs=1) for i in range(NCH)]
        proj_fm("sa_wk", xkv, KV, ktl, evict_copy)
        vts = []
        proj_v(xkv, "sa_wv", KV, vts, sasb, "sa")
        proj_fm("sa_wq", xkv, 512, qt, evict_copy, coff=QOFF)
        at = [trp.tile([P, 512], bf16, name=f"sa_at{i}", tag=f"at{i}",
                       bufs=2) for i in range(NCH)]
        at8 = mk_at8(trp, "sa_at8")
        fin_sa = attention(qt, ktl, vts, at, at8, None, True, "sa")

        # hoist: cc/ck K+V projections are independent of LN1; the cc_wk
        # projection also fills the sa-normalizer scalar chain
        load_w(ccsb, ["cc_wk", "cc_wv", "ck_wk", "ck_wv",
                      "cc_wq", "ck_wq", "cc_wo", "ck_wo"])
        srcl = load_act(ccsb, "src", srcT, kts_cc * P)
        kwe = load_act(ccsb, "kw", kwT, KW)
        cc_kt = [ccsb.tile([P, kts_cc * P], bf16, name=f"cc_k{i}",
                           tag=f"cc_k{i}", bufs=1) for i in range(NCH)]
        proj_fm("cc_wk", srcl, kts_cc * P, cc_kt, evict_copy)
        fin_sa()
        cc_vts = []
        proj_v(srcl, "cc_wv", kts_cc * P, cc_vts, ccsb, "cc")

        def evict_resid_x(j, cs, ce, ps, out_tiles):
            nc.vector.scalar_tensor_tensor(out_tiles[j][:, cs:ce], ps, WINV,
                                           xq[j], op0=OP.mult, op1=OP.add)
        tap("sa_at", at)
        proj_fm("sa_wo", at8, 512, r1, evict_resid_x)
        tap("r1", r1)
        ck_kt = [ccsb.tile([P, KW], bf16, name=f"ck_k{i}", tag=f"ck_k{i}",
                           bufs=1) for i in range(NCH)]
        proj_fm("ck_wk", kwe, KW, ck_kt, evict_copy)
        y8 = [rpool.tile([P, 1024], f8, name=f"y8_{i}", tag=f"lnA8_{i}",
                         bufs=1) for i in range(2)]
        ln1 = layernorm(r1, y, 0, out8=y8)
        ck_vts = []
        proj_v(kwe, "ck_wv", KW, ck_vts, ccsb, "ck")
        ln1()
        tap("y", y)
        sasb_cm.__exit__(None, None, None)

        # --- stage 2: cc + ck cross attention + gate ---
        cc_qt = [ccsb.tile([P, 512], bf16, name=f"cc_q{i}", tag=f"cc_q{i}",
                           bufs=1) for i in range(NCH)]
        proj_fm("cc_wq", y8, 512, cc_qt, evict_copy)
        cc_at = [trp.tile([P, 512], bf16, name=f"cc_at{i}", tag=f"at{i}",
                          bufs=2) for i in range(NCH)]
        cc_at8 = mk_at8(trp, "cc_at8")
        fin_cc = attention(cc_qt, cc_kt, cc_vts, cc_at, cc_at8, ccbias,
                           False, "cc")
        # ck q-projection fills cc's normalize tail
        ck_qt = [ccsb.tile([P, 512], bf16, name=f"ck_q{i}", tag=f"ck_q{i}",
                           bufs=1) for i in range(NCH)]
        proj_fm("ck_wq", y8, 512, ck_qt, evict_copy)
        fin_cc()
        ck_at = [trp.tile([P, 512], bf16, name=f"ck_at{i}", tag=f"ckat{i}",
                          bufs=1) for i in range(NCH)]
        ck_at8 = [trp.tile([P, 1024], f8, name=f"ck_at8_{i}",
                           tag=f"ckat8_{i}", bufs=1) for i in range(2)]
        fin_ck = attention(ck_qt, ck_kt, ck_vts, ck_at, ck_at8, kwbias,
                           False, "ck")
        # ct weight/act loads + K projection fill ck's normalize tail
        ctsb = ctx.enter_context(tc.tile_pool(name="tail_sb", bufs=1))
        load_w(ctsb, ["ct_wk", "ct_wv", "ct_wq", "ct_wo"])
        tmpl = load_act(ctsb, "tmpl", tmplT, kts_ct * P)
        ct_kt = [ctsb.tile([P, kts_ct * P], bf16, name=f"ct_k{i}",
                           tag=f"ct_k{i}", bufs=1) for i in range(NCH)]
        proj_fm("ct_wk", tmpl, kts_ct * P, ct_kt, evict_copy)
        fin_ck()

        # --- gate logits straight off the attention outputs (gwa_t/gwb_t
        # hold the host-fused u = W_o @ gate_w vectors), so the sigmoid
        # chain overlaps the W_o projections ---
        ps_g = psB.tile([1, 512], mybir.dt.float32, name="gate_ps",
                        tag="x_po")
        for i in range(NCH):
            nc.tensor.matmul(ps_g, gwa_t[:, i:i + 1],
                             pair_view(cc_at8[i // 2])[:, i % 2, :],
                             start=(i == 0), stop=False)
        for i in range(NCH):
            nc.tensor.matmul(ps_g, gwb_t[:, i:i + 1],
                             pair_view(ck_at8[i // 2])[:, i % 2, :],
                             start=False, stop=(i == NCH - 1))
        ge = smallp.tile([1, 512], mybir.dt.float32, name="gate_e",
                         tag="gate_edg", bufs=3)
        nc.scalar.activation(ge, ps_g, AF.Exp, scale=-1.0 / GSCALE,
                             bias=gb_t[:, :])
        gl2 = smallp.tile([1, 512], mybir.dt.float32, name="gate_lnd",
                          tag="gate_edg", bufs=3)
        nc.scalar.activation(gl2, ge, AF.Ln, bias=1.0)
        gg = smallp.tile([1, 512], bf16, name="gate_g",
                         tag="gate_edg", bufs=3)
        nc.scalar.activation(gg, gl2, AF.Exp, scale=-1.0)
        # first ct V-projection tile fills the gate sigmoid chain; the
        # rest interleave with the vector-bound blend loop below
        ct_vts = []
        ct_v_nch = (kts_ct * P + P - 1) // P

        def ct_v_tile(m):
            rows = min(P, kts_ct * P - m * P)
            ps = psA.tile([rows, D], mybir.dt.float32, name="v_ps",
                          tag="pps")
            for ip in range(2):
                nc.tensor.matmul(
                    ps, pair_view(tmpl[ip])[:, :, m * P:m * P + rows],
                    w_rhs("ct_wv", ip),
                    start=(ip == 0), stop=(ip == 1), perf_mode=DR)
            if m % 2 == 0:
                nsub = 2 if m + 1 < ct_v_nch else 1
                vtp = ctsb.tile([rows, nsub * H * (DH + 2)], f8,
                                name=f"ct_v{m // 2}", tag=f"ct_v{m // 2}",
                                bufs=1)
                ct_vts.append((vtp, nsub, rows))
            vtp, nsub, _ = ct_vts[-1]
            v4_ = vtp.rearrange("p (two g c) -> p two g c",
                                two=nsub, c=DH + 2)
            nc.any.tensor_scalar_mul(
                v4_[:, m % 2, :, 0:DH],
                ps.rearrange("p (g c) -> p g c", c=DH), WINV)
            nc.vector.memset(v4_[:, m % 2, :, DH:DH + 2], 1.0)

        ct_v_tile(0)
        ggb = psB.tile([P, 512], mybir.dt.float32, name="gate_gb",
                       tag="x_po")
        nc.tensor.matmul(ggb, ones_row, gg, start=True, stop=True)
        # fused blended W_o projections: r2 = y + y2k + g*(y2c - y2k)
        for j in range(NCH):
            if j + 1 < ct_v_nch:
                ct_v_tile(j + 1)
            psc = psA.tile([P, 512], mybir.dt.float32, name="wo_psc",
                           tag="pps")
            for ip in range(2):
                nc.tensor.matmul(psc, w_lhsT("cc_wo", ip, j),
                                 pair_view(cc_at8[ip]),
                                 start=(ip == 0), stop=(ip == 1),
                                 perf_mode=DR)
            psk = psA.tile([P, 512], mybir.dt.float32, name="wo_psk",
                           tag="pps")
            for ip in range(2):
                nc.tensor.matmul(psk, w_lhsT("ck_wo", ip, j),
                                 pair_view(ck_at8[ip]),
                                 start=(ip == 0), stop=(ip == 1),
                                 perf_mode=DR)
            y2ks = trp.tile([P, 512], bf16, name="gate_y2k", tag="gate_y2k",
                            bufs=2)
            nc.scalar.activation(y2ks, psk, AF.Copy, scale=WINV)
            dt_ = trp.tile([P, 512], bf16, name="gate_dt", tag="gate_dt",
                           bufs=2)
            nc.vector.scalar_tensor_tensor(dt_, psc, WINV, y2ks,
                                           op0=OP.mult, op1=OP.subtract)
            nc.vector.tensor_mul(dt_, dt_, ggb)
            nc.vector.tensor_add(r2[j], y[j], y2ks)
            nc.vector.tensor_add(r2[j], r2[j], dt_)
        tap("r2", r2)
        w1t = ctsb.tile([P, NCH * DFF], bf16, name="w1_t", tag="w1_t")
        nc.sync.dma_start(out=w1t.rearrange("p (i n) -> p i n", n=DFF),
                          in_=w1bd.rearrange("(i p) n -> p i n", p=P))
        w2t = ctsb.tile([P, (DFF // P) * D], bf16, name="w2_t", tag="w2_t")
        nc.sync.dma_start(out=w2t.rearrange("p (i n) -> p i n", n=D),
                          in_=w2bd.rearrange("(i p) n -> p i n", p=P))
        z8 = [rpool.tile([P, 1024], f8, name=f"z8_{i}", tag=f"lnB8_{i}",
                         bufs=1) for i in range(2)]
        ln2 = layernorm(r2, z, 1, out8=z8)
        ln2()
        tap("z", z)

        # --- stage 3: ct cross attention ---
        r3 = mktiles("r3", dt=bf16, tagp="rA")
        ze = mktiles("ze", dt=bf16, tagp="lnA")
        ct_qt = [ctsb.tile([P, 512], bf16, name=f"ct_q{i}", tag=f"ct_q{i}",
                           bufs=1) for i in range(NCH)]
        proj_fm("ct_wq", z8, 512, ct_qt, evict_copy)
        ct_at = [trp.tile([P, 512], bf16, name=f"ct_at{i}", tag=f"at{i}",
                          bufs=2) for i in range(NCH)]
        ct_at8 = mk_at8(trp, "ct_at8")
        fin_ct = attention(ct_qt, ct_kt, ct_vts, ct_at, ct_at8, ctbias,
                           False, "ct")
        fin_ct()

        def evict_resid_r2(j, cs, ce, ps, out_tiles):
            nc.vector.scalar_tensor_tensor(out_tiles[j][:, cs:ce], ps, WINV,
                                           z[j], op0=OP.mult, op1=OP.add)
        tap("ct_at", ct_at)
        proj_fm("ct_wo", ct_at8, 512, r3, evict_resid_r2)
        tap("r3", r3)
        ln3 = layernorm(r3, ze, 2)
        ln3()
        tap("ze", ze)

        # --- stage 4: FFN (bf16 for accuracy) ---
        ffsb = ctx.enter_context(tc.tile_pool(name="ff_sb", bufs=1))
        ht = [ffsb.tile([P, 512], bf16, name=f"ff_h{i}", tag=f"ff_h{i}",
                        bufs=1) for i in range(DFF // P)]
        for jf in range(DFF // P):
            ps = psA.tile([P, 512], mybir.dt.float32, name="ff_ps",
                          tag="pps")
            for i in range(NCH):
                nc.tensor.matmul(ps, w1t[:, i * DFF + jf * P:
                                         i * DFF + (jf + 1) * P],
                                 ze[i], start=(i == 0), stop=(i == NCH - 1))
            if jf % 2 == 0:
                nc.scalar.activation(ht[jf], ps, AF.Relu)
            else:
                nc.vector.tensor_scalar_max(ht[jf], ps, 0.0)
        r4 = mktiles("r4", dt=bf16, tagp="rB")
        for j in range(NCH):
            ps = psA.tile([P, 512], mybir.dt.float32, name="ff_ps2",
                          tag="pps")
            for i in range(DFF // P):
                nc.tensor.matmul(ps, w2t[:, i * D + j * P: i * D + (j + 1) * P],
                                 ht[i], start=(i == 0),
                                 stop=(i == DFF // P - 1))
            nc.vector.tensor_add(r4[j], ps, ze[j])
        fin = [trp.tile([P, 512], mybir.dt.float32, name=f"fin{i}",
                        tag=f"fin{i}", bufs=1) for i in range(NCH)]
        ln4 = layernorm(r4, fin, 3)
        ln4()
        for j in range(NCH):
            nc.sync.dma_start(out=outT[j * P:(j + 1) * P, :], in_=fin[j])

    nc.compile()
    return nc


# ---------------------------------------------------------------------------
# host-side input preparation
# ---------------------------------------------------------------------------

def _prep_shared(inputs):
    """Cast/transform weights shared by every core."""
    sh = {}
    for n in ("sa", "cc", "ct", "ck"):
        for p in ("wq", "wk", "wv", "wo"):
            sh[f"{n}_{p}"] = np.ascontiguousarray(
                (inputs[f"{n}_{p}"].astype(F32) * WSCALE).astype(F8))
    sh["ffn_w1b"] = np.ascontiguousarray(inputs["ffn_w1"].astype(BF16))
    sh["ffn_w2b"] = np.ascontiguousarray(inputs["ffn_w2"].astype(BF16))
    gw = inputs["gate_w"].astype(F32)
    # fold the W_o projections into the gate vectors: the gate logit is
    # computed directly from the attention outputs as
    # u_cc^T cc_at + u_ck^T ck_at with u = W_o @ gate_w
    sh["gwA"] = np.ascontiguousarray(
        (inputs["cc_wo"].astype(F32) @ gw[:D] * GSCALE).astype(F8))
    sh["gwB"] = np.ascontiguousarray(
        (inputs["ck_wo"].astype(F32) @ gw[D:] * GSCALE).astype(F8))
    kl, ql = np.arange(P)[:, None], np.arange(P)[None, :]
    sh["stair"] = np.where(kl <= ql, 0.0, np.float32(-65000.0)).astype(BF16)
    sel8 = np.zeros((8, 4 * P), BF16)
    for hp in range(4):
        sel8[2 * hp, hp * P:hp * P + 64] = 1.0
        sel8[2 * hp + 1, hp * P + 64:(hp + 1) * P] = 1.0
    sh["sel8"] = sel8
    return sh


def _len_bias(L, kts, width=P):
    """[width,1] f32 additive bias for the LAST kv tile."""
    base = (kts - 1) * P
    idx = base + np.arange(width)
    return np.where(idx < L, 0.0, NEG).astype(F32)[:, None]


def _prep_core(inputs, sh, b, qh, kts_cc, kts_ct):
    KVn = 512 * (qh + 1)
    QOFF = qh * 512
    m = dict(sh)
    xT = np.ascontiguousarray(inputs["x"][b].T.astype(F32))  # [D, T]
    m["xkvT"] = np.ascontiguousarray(xT[:, :KVn].astype(F8))
    m["xqT"] = np.ascontiguousarray(
        xT[:, QOFF:QOFF + 512].astype(BF16))
    Ls = int(inputs["source_code_len"][b])
    st = np.zeros((D, kts_cc * P), F8)
    st[:, :Ls] = inputs["source_code_enc"][b, :Ls].T.astype(F8)
    m["srcT"] = st
    Lt = int(inputs["template_len"][b])
    tt = np.zeros((D, kts_ct * P), F8)
    tt[:, :Lt] = inputs["template_enc"][b, :Lt].T.astype(F8)
    m["tmplT"] = tt
    m["kwT"] = np.ascontiguousarray(inputs["keywords_enc"][b].T.astype(F8))
    m["cc_bias"] = _len_bias(Ls, kts_cc)
    m["ct_bias"] = _len_bias(Lt, kts_ct)
    m["kw_bias"] = _len_bias(int(inputs["keywords_len"][b]), 1, KW)
    return m


# ---------------------------------------------------------------------------
# concurrent multi-program PJRT runner (adapted from bass2jax.run_bass_via_pjrt)
# ---------------------------------------------------------------------------

def _run_groups(groups):
    """groups: list of (nc, core_ids, in_maps).  Dispatch all groups onto
    their own device subsets, then gather.  Returns {core_id: {name: arr}}."""
    import jax
    import numpy as _np
    from jax.sharding import Mesh, PartitionSpec
    from jax.experimental.shard_map import shard_map
    from concourse import bass2jax
    from concourse.bass2jax import (_bass_exec_p, install_neuronx_cc_hook,
                                    partition_id_tensor)

    install_neuronx_cc_hook()
    devices = jax.devices()

    def make_launch(nc, core_ids, in_maps):
        pname = (nc.partition_id_tensor.name
                 if nc.partition_id_tensor else None)
        in_names, out_names, out_avals, zero_outs = [], [], [], []
        for alloc in nc.m.functions[0].allocations:
            if not isinstance(alloc, mybir.MemoryLocationSet):
                continue
            name = alloc.memorylocations[0].name
            if alloc.kind == "ExternalInput":
                if name == pname:
                    continue
                in_names.append(name)
            elif alloc.kind == "ExternalOutput":
                shape = tuple(alloc.tensor_shape)
                dtype = mybir.dt.np(alloc.dtype)
                out_names.append(name)
                out_avals.append(jax.core.ShapedArray(shape, dtype))
                zero_outs.append(_np.zeros(shape, dtype))
        n_params, n_outs = len(in_names), len(out_avals)
        all_in_names = in_names + out_names
        if pname is not None:
            all_in_names = all_in_names + [pname]

        def _body(*args):
            operands = list(args)
            if pname is not None:
                operands.append(partition_id_tensor())
            outs = _bass_exec_p.bind(
                *operands, out_avals=tuple(out_avals),
                in_names=tuple(all_in_names), out_names=tuple(out_names),
                lowering_input_output_aliases=(),
                sim_require_finite=False, sim_require_nnan=False, nc=nc)
            return tuple(outs)

        donate = tuple(range(n_params, n_params + n_outs))
        devs = [devices[c] for c in core_ids]
        if len(core_ids) == 1:
            fn = jax.jit(_body, donate_argnums=donate, keep_unused=True,
                         device=devs[0])
            args = [in_maps[0][nm] for nm in in_names] + list(zero_outs)
            out_arrs = fn(*args)
            return out_names, out_avals, out_arrs, None
        mesh = Mesh(_np.asarray(devs), ("core",))
        in_specs = (PartitionSpec("core"),) * (n_params + n_outs)
        out_specs = (PartitionSpec("core"),) * n_outs
        fn = jax.jit(shard_map(_body, mesh=mesh, in_specs=in_specs,
                               out_specs=out_specs, check_rep=False),
                     donate_argnums=donate, keep_unused=True)
        cat = [_np.concatenate([_np.asarray(m[nm]) for m in in_maps], axis=0)
               for nm in in_names]
        catz = [_np.zeros((len(core_ids) * z.shape[0], *z.shape[1:]), z.dtype)
                for z in zero_outs]
        out_arrs = fn(*cat, *catz)
        return out_names, out_avals, out_arrs, len(core_ids)

    last_err = None
    for _attempt in range(3):
        try:
            launched = []
            for nc, core_ids, in_maps in groups:
                launched.append((core_ids, make_launch(nc, core_ids, in_maps)))
            results = {}
            for core_ids, (out_names, out_avals, out_arrs, ncores) in launched:
                if ncores is None:
                    results[core_ids[0]] = {nm: _np.asarray(out_arrs[i])
                                            for i, nm in enumerate(out_names)}
                else:
                    for ci, c in enumerate(core_ids):
                        results[c] = {
                            nm: _np.asarray(out_arrs[i]).reshape(
                                ncores, *out_avals[i].shape)[ci]
                            for i, nm in enumerate(out_names)}
            return results
        except Exception as e:  # transient NRT device errors: retry
            last_err = e
            import time as _time
            _time.sleep(2.0)
    raise last_err


_PROGRAM_CACHE = {}
_CACHE_LOCK = threading.Lock()


def _get_program(key):
    with _CACHE_LOCK:
        if key in _PROGRAM_CACHE:
            return _PROGRAM_CACHE[key]
    qh, kts_cc, kts_ct, gate_b, aff = key
    nc = build_program(qh, kts_cc, kts_ct, gate_b=gate_b, apply_affine=aff)
    with _CACHE_LOCK:
        _PROGRAM_CACHE[key] = nc
    return nc


# ---------------------------------------------------------------------------
# entry point
# ---------------------------------------------------------------------------

def kernel(**inputs):
    inputs = {k: np.asarray(v) for k, v in inputs.items()}
    gate_b = float(inputs["gate_b"].reshape(-1)[0])
    aff = not all(
        np.all(inputs[f"ln{j}_g"] == 1.0) and np.all(inputs[f"ln{j}_b"] == 0.0)
        for j in range(1, 5))
    affine_arr = None
    if aff:
        affine_arr = np.zeros((P, NCH * 8), F32)
        for ln in range(4):
            g = inputs[f"ln{ln + 1}_g"].astype(F32).reshape(NCH, P).T
            bb = inputs[f"ln{ln + 1}_b"].astype(F32).reshape(NCH, P).T
            affine_arr[:, ln * 2 * NCH: ln * 2 * NCH + NCH] = g
            affine_arr[:, ln * 2 * NCH + NCH: (ln + 1) * 2 * NCH] = bb

    sh = _prep_shared(inputs)
    # core -> (program key, in_map)
    core_keys, core_maps = [], []
    for c in range(8):
        b, qh = c // 2, c % 2
        kts_cc = max(1, -(-int(inputs["source_code_len"][b]) // P))
        kts_ct = max(1, -(-int(inputs["template_len"][b]) // P))
        key = (qh, kts_cc, kts_ct, gate_b, aff)
        m = _prep_core(inputs, sh, b, qh, kts_cc, kts_ct)
        if aff:
            m["ln_affine"] = affine_arr
        core_keys.append(key)
        core_maps.append(m)

    # build distinct programs (parallel threads: walrus compile is subprocess)
    distinct = sorted(set(core_keys))
    threads = [threading.Thread(target=_get_program, args=(k,))
               for k in distinct]
    for t in threads:
        t.start()
    for t in threads:
        t.join()

    groups = []
    for key in distinct:
        cores = [c for c in range(8) if core_keys[c] == key]
        groups.append((_get_program(key), cores, [core_maps[c] for c in cores]))

    results = _run_groups(groups)

    out = np.empty((B, T, D), np.float32)
    for c in range(8):
        b, qh = c // 2, c % 2
        out[b, qh * 512:(qh + 1) * 512, :] = results[c]["outT"].T
    return out



# revision 27
# speedup vs baseline: 1.0405x; 1.0405x over previous
"""Trainium2 Bass kernel for nn_DecoderBlockWithKeywords.

Decoder block: causal self-attn + gated (source-code / keywords) cross-attn
+ template cross-attn + FFN, with 4 LayerNorms.  B=4, T=1024, D=512, H=8,
dh=64, DFF=2048.

Sharding: pure data-parallel over (batch, query-half) -> 8 NeuronCores, no
collectives.  Each core holds all weights (fp16) and computes 512 query
tokens of one batch element.

Layout strategy: every activation lives feature-major (X^T: [D on
partitions, tokens on free]).  Host pre-transposes/casts inputs.  Q/K
projections are weight-stationary (out feature-major); V is produced
token-major via activation-stationary matmuls so the attention AV matmul
needs no transposes at all.  Scores are computed as S^T = K_h Q_h^T
([kv, q]); softmax runs without max-subtraction (logits are O(1); masked
lanes get -1e6 bias fused into the ACT exp).  Softmax denominators come
from a ones-column appended to V inside the same AV matmul; per-column
scales (softmax 1/n, LN mean/rstd, gate g) are broadcast across partitions
with a PE ones-outer-product into a free PSUM bank and applied by one DVE
op reading that PSUM operand.  LayerNorm is done feature-major: column sums
via PE ones-matmuls, rstd = exp(-0.5*ln(v)) on ACT (single activation-table
set, zero table switches).  Residuals follow the reference post-LN chaining
(z = LN(y + y2), z_end = LN(z + z2), out = LN(z_end + ff)).

Programs are specialized at build time to the actual kv lengths (read from
the int32 length inputs), so masked kv tiles are skipped entirely; up to 8
distinct programs (4 batches x even/odd query half) are compiled and
launched concurrently on disjoint device subsets.
"""

import os
import sys
import threading

import numpy as np

for _p in ("/opt/trn_rl_repo", "/root/.axon_site"):
    if os.path.isdir(_p) and _p not in sys.path:
        sys.path.append(_p)

import ml_dtypes
from contextlib import ExitStack

import concourse.bass as bass
import concourse.mybir as mybir
from concourse import bacc
from concourse.tile import TileContext

BF16 = np.float16
F32 = np.float32
F8 = ml_dtypes.float8_e4m3
WSCALE = 16.0      # weights stored as w*16 in fp8 (values ~N(0,0.02))
WINV = 1.0 / WSCALE
GSCALE = 64.0      # gate u-vectors stored as u*64 in fp8
NEG = -1000000.0
B, T, S, TM, KW, D, H, DFF = 4, 1024, 1024, 512, 64, 512, 8, 2048
DH = D // H  # 64
P = 128
NCH = D // P  # 4 feature chunks
AF = mybir.ActivationFunctionType
OP = mybir.AluOpType


# ---------------------------------------------------------------------------
# program builder
# ---------------------------------------------------------------------------

def build_program(qh, kts_cc, kts_ct, gate_b=0.0, apply_affine=False, debug=False):
    """Build one core's Bass program.

    qh: 0/1 query half.  kts_cc/kts_ct: number of 128-wide kv tiles for the
    source-code / template cross attentions (specialized to actual length).
    """
    f32, bf16 = mybir.dt.float32, mybir.dt.float16
    f8 = mybir.dt.float8e4
    DR = mybir.MatmulPerfMode.DoubleRow
    KV = 512 * (qh + 1)          # self-attn kv range
    QOFF = qh * 512              # q columns inside xkvT

    nc = bacc.Bacc("TRN2", target_bir_lowering=False, debug=False)

    def din(name, shape, dt=bf16):
        return nc.dram_tensor(name, shape, dt, kind="ExternalInput").ap()

    xkvT = din("xkvT", [D, KV], f8)
    xqT = din("xqT", [D, 512])       # bf16 x for the residual add
    srcT = din("srcT", [D, kts_cc * P], f8)
    tmplT = din("tmplT", [D, kts_ct * P], f8)
    kwT = din("kwT", [D, KW], f8)
    wnames = [f"{n}_{p}" for n in ("sa", "cc", "ct", "ck")
              for p in ("wq", "wk", "wv", "wo")]
    wd = {n: din(n, [D, D], f8) for n in wnames}
    w1bd = din("ffn_w1b", [D, DFF])
    w2bd = din("ffn_w2b", [DFF, D])
    gwA = din("gwA", [D, 1], f8)
    gwB = din("gwB", [D, 1], f8)
    staird = din("stair", [P, P])
    sel8_d = din("sel8", [H, NCH * P])
    ccbias_d = din("cc_bias", [P, 1], f32)
    ctbias_d = din("ct_bias", [P, 1], f32)
    kwbias_d = din("kw_bias", [KW, 1], f32)
    affine_d = din("ln_affine", [P, NCH * 8], f32) if apply_affine else None
    outT = nc.dram_tensor("outT", [D, 512], f32, kind="ExternalOutput").ap()
    dbg_outs = {}

    def mkdbg(nm, shape):
        if nm not in dbg_outs:
            dbg_outs[nm] = nc.dram_tensor(f"dbg_{nm}", shape, f32,
                                          kind="ExternalOutput").ap()
        return dbg_outs[nm]

    with TileContext(nc, pool_alloc_mode="queue") as tc, ExitStack() as ctx:
        # Pre-place one ACT table covering Exp+Ln+Copy+Square+Relu: without
        # this the compiler's greedy chooser ping-pongs exp_and_others <->
        # natural_log (~19 loads x 1.3us on the critical path).
        from concourse.hw_specs import get_activation_tables
        _tabs = list(get_activation_tables(nc.m.arch).keys())
        nc.scalar.add_instruction(mybir.InstLoadActFuncSet(
            act_func_set_id=_tabs.index("natural_log_exp_and_others"),
            name=nc.get_next_instruction_name(),
            engine=mybir.EngineType.Activation))
        pers = ctx.enter_context(tc.tile_pool(name="pers", bufs=1))
        # ---- persistent small constants -------------------------------
        # (const DMAs are emitted by emit_consts() AFTER the stage-1
        # critical weight/activation loads so they don't delay the first
        # projection)
        stair = pers.tile([P, P], bf16, name="stair_t")
        ccbias = pers.tile([P, 1], f32, name="ccbias_t")
        ctbias = pers.tile([P, 1], f32, name="ctbias_t")
        kwbias = pers.tile([KW, 1], f32, name="kwbias_t")
        gwa_t = pers.tile([P, NCH], f8, name="gwa_t")
        gwb_t = pers.tile([P, NCH], f8, name="gwb_t")
        sel8 = pers.tile([H, NCH * P], bf16, name="sel8_t")
        affine = None
        if apply_affine:
            affine = pers.tile([P, NCH * 8], f32, name="affine_t")

        def emit_consts():
            nc.sync.dma_start(out=stair, in_=staird)
            nc.sync.dma_start(out=ccbias, in_=ccbias_d)
            nc.sync.dma_start(out=ctbias, in_=ctbias_d)
            nc.sync.dma_start(out=kwbias, in_=kwbias_d)
            nc.sync.dma_start(out=gwa_t,
                              in_=gwA.rearrange("(i p) o -> p i o", p=P))
            nc.sync.dma_start(out=gwb_t,
                              in_=gwB.rearrange("(i p) o -> p i o", p=P))
            nc.sync.dma_start(out=sel8, in_=sel8_d)
            if apply_affine:
                nc.sync.dma_start(out=affine, in_=affine_d)

        ones_f = pers.tile([P, 1], f32, name="ones_f")
        nc.vector.memset(ones_f, 1.0)
        ones_b = pers.tile([P, 1], bf16, name="ones_b")
        nc.vector.memset(ones_b, 1.0)
        ones_row = pers.tile([1, P], bf16, name="ones_row")
        nc.vector.memset(ones_row, 1.0)
        eps_t = pers.tile([1, 1], f32, name="eps_t")
        nc.vector.memset(eps_t, 1e-5)
        gb_t = pers.tile([1, 1], f32, name="gb_t")
        nc.vector.memset(gb_t, -float(gate_b))

        def tap(nm, tiles):
            if not debug:
                return
            cols = tiles[0].shape[-1]
            d = mkdbg(nm, [len(tiles) * P, cols])
            for i, t in enumerate(tiles):
                rows = t.shape[0]
                nc.gpsimd.dma_start(out=d[i * P:i * P + rows, :], in_=t)

        # ---- global shared pools --------------------------------------
        # residual/LN-out tiles, reused across stages via shared tags
        rpool = ctx.enter_context(tc.tile_pool(name="rpool", bufs=1))

        def mktiles(nm, cols=512, dt=f32, n=NCH, tagp=None):
            tagp = tagp or nm
            return [rpool.tile([P, cols], dt, name=f"{nm}{i}", tag=f"{tagp}{i}",
                               bufs=1) for i in range(n)]

        # small 1/8-partition tiles + broadcast tiles, shared by all stages
        smallp = ctx.enter_context(tc.tile_pool(name="smallp", bufs=1))
        # transient [128, *] tiles (exp outputs, LN scratch, gate scratch)
        trp = ctx.enter_context(tc.tile_pool(name="trp", bufs=1))
        # PSUM: pps = projection/V accumulators; x_ps = paired scores
        # (2 banks each); x_po = AV out + LN stats + gate
        psA = ctx.enter_context(tc.tile_pool(name="psA", bufs=2, space="PSUM"))
        psB = ctx.enter_context(tc.tile_pool(name="psB", bufs=2, space="PSUM"))

        def load_w(pool, names):
            # fp8 weights in DoubleRow-paired layout: free dims
            # (ipair, two, out-col)
            for n in names:
                wt[n] = pool.tile([P, NCH * D], f8, name=f"{n}_t",
                                  tag=f"{n}_t", bufs=1)
                nc.sync.dma_start(
                    out=wt[n].rearrange("p (ip two n) -> p ip two n",
                                        ip=2, two=2),
                    in_=wd[n].rearrange("(ip two p) n -> p ip two n",
                                        p=P, two=2))
        wt = {}

        def w_lhsT(n, ip, j):
            # [128, 2, 128] stationary pair for DoubleRow
            return wt[n].rearrange("p (ip two n) -> p ip two n",
                                   ip=2, two=2)[:, ip, :, j * P:(j + 1) * P]

        def w_rhs(n, ip, cols=D):
            # [128, 2, cols] moving pair for DoubleRow
            return wt[n].rearrange("p (ip two n) -> p ip two n",
                                   ip=2, two=2)[:, ip, :, 0:cols]

        def load_act(pool, nm, dram_ap, cols):
            # fp8 activations as 2 chunk-paired tiles [P, 2, cols]
            tiles = []
            for pi in range(2):
                t = pool.tile([P, 2 * cols], f8, name=f"{nm}{pi}",
                              tag=f"{nm}{pi}", bufs=1)
                nc.sync.dma_start(
                    out=t.rearrange("p (two n) -> p two n", two=2),
                    in_=dram_ap[pi * 2 * P:(pi + 1) * 2 * P, :].rearrange(
                        "(two p) n -> p two n", p=P))
                tiles.append(t)
            return tiles

        def pair_view(t):
            return t.rearrange("p (two n) -> p two n", two=2)

        # ----------------------------------------------------------------
        # helpers
        # ----------------------------------------------------------------
        def proj_fm(wn, rhs_pairs, ncols, out_tiles, evict, coff=0):
            # rhs_pairs: 2 chunk-paired fp8 tiles [P, 2, >=coff+ncols]
            ntt = (ncols + 511) // 512
            for j in range(NCH):
                for t in range(ntt):
                    cs = t * 512
                    ce = min(ncols, cs + 512)
                    ps = psA.tile([P, ce - cs], mybir.dt.float32,
                                  name="proj_ps", tag="pps")
                    for ip in range(2):
                        nc.tensor.matmul(
                            ps, w_lhsT(wn, ip, j),
                            pair_view(rhs_pairs[ip])[:, :,
                                                     coff + cs:coff + ce],
                            start=(ip == 0), stop=(ip == 1),
                            perf_mode=DR)
                    evict(j, cs, ce, ps, out_tiles)

        def evict_copy(j, cs, ce, ps, out_tiles):
            nc.vector.tensor_scalar_mul(out_tiles[j][:, cs:ce], ps, WINV)

        def proj_v(enc_pairs, wn, nkv, vt_list, vpool, ktag):
            # produces kv-tile-PAIRED fp8 V tiles [rows, nsub, H*(DH+1)]
            # (nsub=2 except a trailing odd tile) for DoubleRow AV matmuls
            nch_tok = (nkv + P - 1) // P
            for m in range(nch_tok):
                rows = min(P, nkv - m * P)
                ps = psA.tile([rows, D], mybir.dt.float32,
                              name="v_ps", tag="pps")
                for ip in range(2):
                    nc.tensor.matmul(
                        ps, pair_view(enc_pairs[ip])[:, :, m * P:m * P + rows],
                        w_rhs(wn, ip),
                        start=(ip == 0), stop=(ip == 1), perf_mode=DR)
                if m % 2 == 0:
                    nsub = 2 if m + 1 < nch_tok else 1
                    vtp = vpool.tile([rows, nsub * H * (DH + 2)], f8,
                                     name=f"{ktag}_v{m // 2}",
                                     tag=f"{ktag}_v{m // 2}", bufs=1)
                    vt_list.append((vtp, nsub, rows))
                vtp, nsub, _ = vt_list[-1]
                v4 = vtp.rearrange("p (two g c) -> p two g c",
                                   two=nsub, c=DH + 2)
                nc.any.tensor_scalar_mul(
                    v4[:, m % 2, :, 0:DH],
                    ps.rearrange("p (g c) -> p g c", c=DH), WINV)
                nc.vector.memset(v4[:, m % 2, :, DH:DH + 2], 1.0)

        def attention(qt, kt, vt_pairs, out_tiles, out8, bias_tile, causal,
                      ktag):
            """Multi-head attention, head pairs share one [rows,1024]
            scores psum + one merged exp (fp8 out, kv-pair planes).  AV is
            a DoubleRow fp8 matmul over kv-tile pairs.  Causal tiles only
            touch live query columns.  Returns a finish() closure that
            emits the normalizer broadcasts + final muls, so callers can
            interleave independent PE work with the 1/n scalar chain."""
            nkt = sum(ns for _, ns, _ in vt_pairs)
            npairs = len(vt_pairs)
            nmat = smallp.tile([H, 512], mybir.dt.float32,
                               name=f"{ktag}_nmat", tag="nmat", bufs=1)
            for hp in range(H // 2):
                po = []
                for s in range(2):
                    po.append(psB.tile([DH + 2, 512], mybir.dt.float32,
                                       name=f"{ktag}_po{s}", tag="x_po"))
                kt_base = 0
                for mp, (vtp, nsub, vrows) in enumerate(vt_pairs):
                    ds = [(kt_base + u - (nkt - 4)) if causal else -1
                          for u in range(nsub)]
                    c0s = [d * P if (causal and d > 0) else 0 for d in ds]
                    c0p = c0s[0]
                    pp = trp.tile([vrows, nsub * 1024], f8,
                                  name=f"{ktag}_pt", tag="pt", bufs=3)
                    pp3 = pp.rearrange("p (two n) -> p two n", two=nsub)
                    for u in range(nsub):
                        kt_i = kt_base + u
                        c0 = c0s[u]
                        d = ds[u]
                        ps2 = psB.tile([vrows, 1024], mybir.dt.float32,
                                       name=f"{ktag}_ps", tag="x_ps")
                        for s in range(2):
                            ro = s * DH
                            o = s * 512
                            nc.tensor.matmul(
                                ps2[:, o + c0:o + 512],
                                kt[hp][ro:ro + DH,
                                       kt_i * P:kt_i * P + vrows],
                                qt[hp][ro:ro + DH, c0:512],
                                start=True, stop=True)
                        if causal and d >= 0:
                            for s in range(2):
                                o = s * 512
                                nc.vector.tensor_add(
                                    ps2[:, o + c0:o + c0 + P],
                                    ps2[:, o + c0:o + c0 + P], stair)
                                if c0 > c0p:
                                    nc.vector.memset(
                                        pp3[:, u, o + c0p:o + c0], 0.0)
                            # one strided exp covers both heads' live range
                            nc.scalar.activation(
                                pp.rearrange(
                                    "p (two s n) -> p two s n",
                                    two=nsub, s=2)[:, u, :, c0:512],
                                ps2.rearrange("p (s n) -> p s n",
                                              s=2)[:, :, c0:512],
                                AF.Exp, scale=0.125)
                        else:
                            bias = 0.0
                            if bias_tile is not None and kt_i == nkt - 1:
                                bias = bias_tile[:vrows, :]
                            nc.scalar.activation(pp3[:, u, :], ps2, AF.Exp,
                                                 bias=bias, scale=0.125)
                    v4 = vtp.rearrange("p (two g c) -> p two g c",
                                       two=nsub, c=DH + 2)
                    for s in range(2):
                        h = 2 * hp + s
                        o = s * 512
                        if nsub == 2:
                            nc.tensor.matmul(
                                po[s][:, c0p:512], v4[:, :, h, :],
                                pp3[:, :, o + c0p:o + 512],
                                start=(mp == 0), stop=(mp == npairs - 1),
                                skip_group_check=True, perf_mode=DR)
                        else:
                            nc.tensor.matmul(
                                po[s][:, c0p:512], v4[:, 0, h, :],
                                pp3[:, 0, o + c0p:o + 512],
                                start=(mp == 0), stop=(mp == npairs - 1),
                                skip_group_check=True)
                    kt_base += nsub
                for s in range(2):
                    ro = s * DH
                    h = 2 * hp + s
                    nscr = smallp.tile([1, 512], mybir.dt.float32,
                                       name=f"{ktag}_nscr{h}", tag="nscr",
                                       bufs=2)
                    nc.any.tensor_copy(nscr, po[s][DH:DH + 1, :])
                    nc.sync.dma_start(out=nmat[h:h + 1, :], in_=nscr)
                    nc.any.tensor_copy(out_tiles[hp][ro:ro + DH, :],
                                       po[s][0:DH, :])
            lnn = smallp.tile([H, 512], mybir.dt.float32,
                              name=f"{ktag}_lnn", tag="lnn", bufs=1)
            nc.scalar.activation(lnn, nmat, AF.Ln)
            ninv8 = smallp.tile([H, 512], bf16,
                                name=f"{ktag}_ninv8", tag=f"ninv8_{ktag}",
                                bufs=1)
            nc.scalar.activation(ninv8, lnn, AF.Exp, scale=-1.0)

            def finish():
                for hp in range(H // 2):
                    nb = psA.tile([P, 512], mybir.dt.float32,
                                  name=f"{ktag}_nb", tag="pps")
                    nc.tensor.matmul(nb, sel8[:, hp * P:(hp + 1) * P],
                                     ninv8, start=True, stop=True)
                    nc.vector.tensor_mul(
                        pair_view(out8[hp // 2])[:, hp % 2, :],
                        out_tiles[hp], nb)
            return finish

        def layernorm(r_tiles, out_tiles, ln_idx, out8=None):
            sq = [trp.tile([P, 512], bf16, name=f"ln{ln_idx}_sq", tag="ln_sq",
                           bufs=4) for _ in range(NCH)]
            for j in range(NCH):
                nc.scalar.activation(sq[j], r_tiles[j], AF.Square)
            ps_s = psB.tile([1, 512], mybir.dt.float32,
                            name="ln_ps_s", tag="x_po")
            ps_q = psB.tile([1, 512], mybir.dt.float32,
                            name="ln_ps_q", tag="x_po")
            for j in range(NCH):
                nc.tensor.matmul(ps_s, ones_b, r_tiles[j],
                                 start=(j == 0), stop=(j == NCH - 1))
            for j in range(NCH):
                nc.tensor.matmul(ps_q, ones_b, sq[j],
                                 start=(j == 0), stop=(j == NCH - 1))
            mean16 = smallp.tile([1, 512], bf16,
                                 name="ln_mean16", tag="ln_stat", bufs=3)
            nc.vector.tensor_scalar_mul(mean16, ps_s, 1.0 / D)
            msq = smallp.tile([1, 512], mybir.dt.float32,
                              name="ln_msq", tag="ln_stat", bufs=3)
            nc.vector.tensor_mul(msq, mean16, mean16)
            var = smallp.tile([1, 512], mybir.dt.float32,
                              name="ln_var", tag="ln_stat", bufs=3)
            nc.vector.scalar_tensor_tensor(var, ps_q, 1.0 / D, msq,
                                           op0=OP.mult, op1=OP.subtract)
            lnv = smallp.tile([1, 512], mybir.dt.float32,
                              name="ln_lnv", tag="ln_stat", bufs=3)
            nc.scalar.activation(lnv, var, AF.Ln, bias=eps_t[:, :])
            rstd = smallp.tile([1, 512], bf16,
                               name="ln_rstd", tag="ln_stat", bufs=3)
            nc.scalar.activation(rstd, lnv, AF.Exp, scale=-0.5)

            def apply():
                meanb = psB.tile([P, 512], mybir.dt.float32,
                                 name="ln_meanb", tag="x_po")
                nc.tensor.matmul(meanb, ones_row, mean16,
                                 start=True, stop=True)
                rstdb = psB.tile([P, 512], mybir.dt.float32,
                                 name="ln_rstdb", tag="x_po")
                nc.tensor.matmul(rstdb, ones_row, rstd,
                                 start=True, stop=True)
                for j in range(NCH):
                    tmp = trp.tile([P, 512], bf16,
                                   name="ln_tmp", tag="ln_tmp", bufs=2)
                    nc.vector.tensor_sub(tmp, r_tiles[j], meanb)
                    nc.vector.tensor_mul(out_tiles[j], tmp, rstdb)
                    if apply_affine:
                        g = affine[:, ln_idx * 2 * NCH + j:
                                   ln_idx * 2 * NCH + j + 1]
                        b = affine[:, ln_idx * 2 * NCH + NCH + j:
                                   ln_idx * 2 * NCH + NCH + j + 1]
                        nc.vector.tensor_scalar(out_tiles[j], out_tiles[j],
                                                g, b, op0=OP.mult,
                                                op1=OP.add)
                    if out8 is not None:
                        nc.scalar.activation(
                            pair_view(out8[j // 2])[:, j % 2, :],
                            out_tiles[j], AF.Copy)
            return apply

        # ================================================================
        # emission (ordered for cross-stage overlap)
        # ================================================================
        r1 = mktiles("r1", dt=bf16, tagp="rA")
        y = mktiles("y", dt=bf16, tagp="lnA")
        r2 = mktiles("r2", dt=bf16, tagp="rB")
        z = mktiles("z", dt=bf16, tagp="lnB")
        r3 = None  # allocated after r1 dies
        ze = None

        ccsb = ctx.enter_context(tc.tile_pool(name="cc_sb", bufs=1))
        sasb_cm = tc.tile_pool(name="sa_sb", bufs=1)
        sasb = sasb_cm.__enter__()

        def mk_at8(pool, nm):
            return [pool.tile([P, 1024], f8, name=f"{nm}{i}",
                              tag=f"at8_{i}", bufs=2) for i in range(2)]

        # --- stage 1: self attention ---
        load_w(sasb, ["sa_wk"])
        xkv = load_act(sasb, "xkv", xkvT, KV)
        xq = []
        for i in range(NCH):
            t = sasb.tile([P, 512], bf16, name=f"xq{i}", tag=f"xq{i}",
                          bufs=1)
            nc.sync.dma_start(out=t, in_=xqT[i * P:(i + 1) * P, :])
            xq.append(t)
        load_w(sasb, ["sa_wv", "sa_wq", "sa_wo"])
        emit_consts()
        qt = [sasb.tile([P, 512], bf16, name=f"sa_q{i}", tag=f"sa_q{i}",
                        bufs=1) for i in range(NCH)]
        ktl = [sasb.tile([P, KV], bf16, name=f"sa_k{i}", tag=f"sa_k{i}",
                         bufs=1) for i in range(NCH)]
        proj_fm("sa_wk", xkv, KV, ktl, evict_copy)
        vts = []
        proj_v(xkv, "sa_wv", KV, vts, sasb, "sa")
        proj_fm("sa_wq", xkv, 512, qt, evict_copy, coff=QOFF)
        at = [trp.tile([P, 512], bf16, name=f"sa_at{i}", tag=f"at{i}",
                       bufs=2) for i in range(NCH)]
        at8 = mk_at8(trp, "sa_at8")
        fin_sa = attention(qt, ktl, vts, at, at8, None, True, "sa")

        # hoist: cc/ck K+V projections are independent of LN1; the cc_wk
        # projection also fills the sa-normalizer scalar chain
        load_w(ccsb, ["cc_wk", "cc_wv", "ck_wk", "ck_wv",
                      "cc_wq", "ck_wq", "cc_wo", "ck_wo"])
        srcl = load_act(ccsb, "src", srcT, kts_cc * P)
        kwe = load_act(ccsb, "kw", kwT, KW)
        cc_kt = [ccsb.tile([P, kts_cc * P], bf16, name=f"cc_k{i}",
                           tag=f"cc_k{i}", bufs=1) for i in range(NCH)]
        proj_fm("cc_wk", srcl, kts_cc * P, cc_kt, evict_copy)
        fin_sa()
        cc_vts = []
        proj_v(srcl, "cc_wv", kts_cc * P, cc_vts, ccsb, "cc")

        def evict_resid_x(j, cs, ce, ps, out_tiles):
            nc.vector.scalar_tensor_tensor(out_tiles[j][:, cs:ce], ps, WINV,
                                           xq[j], op0=OP.mult, op1=OP.add)
        tap("sa_at", at)
        proj_fm("sa_wo", at8, 512, r1, evict_resid_x)
        tap("r1", r1)
        ck_kt = [ccsb.tile([P, KW], bf16, name=f"ck_k{i}", tag=f"ck_k{i}",
                           bufs=1) for i in range(NCH)]
        proj_fm("ck_wk", kwe, KW, ck_kt, evict_copy)
        y8 = [rpool.tile([P, 1024], f8, name=f"y8_{i}", tag=f"lnA8_{i}",
                         bufs=1) for i in range(2)]
        ln1 = layernorm(r1, y, 0, out8=y8)
        ck_vts = []
        proj_v(kwe, "ck_wv", KW, ck_vts, ccsb, "ck")
        ln1()
        tap("y", y)
        sasb_cm.__exit__(None, None, None)

        # --- stage 2: cc + ck cross attention + gate ---
        cc_qt = [ccsb.tile([P, 512], bf16, name=f"cc_q{i}", tag=f"cc_q{i}",
                           bufs=1) for i in range(NCH)]
        proj_fm("cc_wq", y8, 512, cc_qt, evict_copy)
        cc_at = [trp.tile([P, 512], bf16, name=f"cc_at{i}", tag=f"at{i}",
                          bufs=2) for i in range(NCH)]
        cc_at8 = mk_at8(trp, "cc_at8")
        fin_cc = attention(cc_qt, cc_kt, cc_vts, cc_at, cc_at8, ccbias,
                           False, "cc")
        # ck q-projection fills cc's normalize tail
        ck_qt = [ccsb.tile([P, 512], bf16, name=f"ck_q{i}", tag=f"ck_q{i}",
                           bufs=1) for i in range(NCH)]
        proj_fm("ck_wq", y8, 512, ck_qt, evict_copy)
        fin_cc()
        ck_at = [trp.tile([P, 512], bf16, name=f"ck_at{i}", tag=f"ckat{i}",
                          bufs=1) for i in range(NCH)]
        ck_at8 = [trp.tile([P, 1024], f8, name=f"ck_at8_{i}",
                           tag=f"ckat8_{i}", bufs=1) for i in range(2)]
        fin_ck = attention(ck_qt, ck_kt, ck_vts, ck_at, ck_at8, kwbias,
                           False, "ck")
        # ct weight/act loads + K projection fill ck's normalize tail
        ctsb = ctx.enter_context(tc.tile_pool(name="tail_sb", bufs=1))
        load_w(ctsb, ["ct_wk", "ct_wv", "ct_wq", "ct_wo"])
        tmpl = load_act(ctsb, "tmpl", tmplT, kts_ct * P)
        ct_kt = [ctsb.tile([P, kts_ct * P], bf16, name=f"ct_k{i}",
                           tag=f"ct_k{i}", bufs=1) for i in range(NCH)]
        proj_fm("ct_wk", tmpl, kts_ct * P, ct_kt, evict_copy)
        fin_ck()

        # --- gate logits straight off the attention outputs (gwa_t/gwb_t
        # hold the host-fused u = W_o @ gate_w vectors), so the sigmoid
        # chain overlaps the W_o projections ---
        ps_g = psB.tile([1, 512], mybir.dt.float32, name="gate_ps",
                        tag="x_po")
        for i in range(NCH):
            nc.tensor.matmul(ps_g, gwa_t[:, i:i + 1],
                             pair_view(cc_at8[i // 2])[:, i % 2, :],
                             start=(i == 0), stop=False)
        for i in range(NCH):
            nc.tensor.matmul(ps_g, gwb_t[:, i:i + 1],
                             pair_view(ck_at8[i // 2])[:, i % 2, :],
                             start=False, stop=(i == NCH - 1))
        ge = smallp.tile([1, 512], mybir.dt.float32, name="gate_e",
                         tag="gate_edg", bufs=3)
        nc.scalar.activation(ge, ps_g, AF.Exp, scale=-1.0 / GSCALE,
                             bias=gb_t[:, :])
        gl2 = smallp.tile([1, 512], mybir.dt.float32, name="gate_lnd",
                          tag="gate_edg", bufs=3)
        nc.scalar.activation(gl2, ge, AF.Ln, bias=1.0)
        gg = smallp.tile([1, 512], bf16, name="gate_g",
                         tag="gate_edg", bufs=3)
        nc.scalar.activation(gg, gl2, AF.Exp, scale=-1.0)
        # first ct V-projection tile fills the gate sigmoid chain; the
        # rest interleave with the vector-bound blend loop below
        ct_vts = []
        ct_v_nch = (kts_ct * P + P - 1) // P

        def ct_v_tile(m):
            rows = min(P, kts_ct * P - m * P)
            ps = psA.tile([rows, D], mybir.dt.float32, name="v_ps",
                          tag="pps")
            for ip in range(2):
                nc.tensor.matmul(
                    ps, pair_view(tmpl[ip])[:, :, m * P:m * P + rows],
                    w_rhs("ct_wv", ip),
                    start=(ip == 0), stop=(ip == 1), perf_mode=DR)
            if m % 2 == 0:
                nsub = 2 if m + 1 < ct_v_nch else 1
                vtp = ctsb.tile([rows, nsub * H * (DH + 2)], f8,
                                name=f"ct_v{m // 2}", tag=f"ct_v{m // 2}",
                                bufs=1)
                ct_vts.append((vtp, nsub, rows))
            vtp, nsub, _ = ct_vts[-1]
            v4_ = vtp.rearrange("p (two g c) -> p two g c",
                                two=nsub, c=DH + 2)
            nc.any.tensor_scalar_mul(
                v4_[:, m % 2, :, 0:DH],
                ps.rearrange("p (g c) -> p g c", c=DH), WINV)
            nc.vector.memset(v4_[:, m % 2, :, DH:DH + 2], 1.0)

        ct_v_tile(0)
        ggb = psB.tile([P, 512], mybir.dt.float32, name="gate_gb",
                       tag="x_po")
        nc.tensor.matmul(ggb, ones_row, gg, start=True, stop=True)
        # fused blended W_o projections: r2 = y + y2k + g*(y2c - y2k)
        for j in range(NCH):
            if j + 1 < ct_v_nch:
                ct_v_tile(j + 1)
            psc = psA.tile([P, 512], mybir.dt.float32, name="wo_psc",
                           tag="pps")
            for ip in range(2):
                nc.tensor.matmul(psc, w_lhsT("cc_wo", ip, j),
                                 pair_view(cc_at8[ip]),
                                 start=(ip == 0), stop=(ip == 1),
                                 perf_mode=DR)
            psk = psA.tile([P, 512], mybir.dt.float32, name="wo_psk",
                           tag="pps")
            for ip in range(2):
                nc.tensor.matmul(psk, w_lhsT("ck_wo", ip, j),
                                 pair_view(ck_at8[ip]),
                                 start=(ip == 0), stop=(ip == 1),
                                 perf_mode=DR)
            y2ks = trp.tile([P, 512], bf16, name="gate_y2k", tag="gate_y2k",
                            bufs=2)
            nc.scalar.activation(y2ks, psk, AF.Copy, scale=WINV)
            dt_ = trp.tile([P, 512], bf16, name="gate_dt", tag="gate_dt",
                           bufs=2)
            nc.vector.scalar_tensor_tensor(dt_, psc, WINV, y2ks,
                                           op0=OP.mult, op1=OP.subtract)
            nc.vector.tensor_mul(dt_, dt_, ggb)
            nc.vector.tensor_add(r2[j], y[j], y2ks)
            nc.vector.tensor_add(r2[j], r2[j], dt_)
        tap("r2", r2)
        w1t = ctsb.tile([P, NCH * DFF], bf16, name="w1_t", tag="w1_t")
        nc.sync.dma_start(out=w1t.rearrange("p (i n) -> p i n", n=DFF),
                          in_=w1bd.rearrange("(i p) n -> p i n", p=P))
        w2t = ctsb.tile([P, (DFF // P) * D], bf16, name="w2_t", tag="w2_t")
        nc.sync.dma_start(out=w2t.rearrange("p (i n) -> p i n", n=D),
                          in_=w2bd.rearrange("(i p) n -> p i n", p=P))
        z8 = [rpool.tile([P, 1024], f8, name=f"z8_{i}", tag=f"lnB8_{i}",
                         bufs=1) for i in range(2)]
        ln2 = layernorm(r2, z, 1, out8=z8)
        ln2()
        tap("z", z)

        # --- stage 3: ct cross attention ---
        r3 = mktiles("r3", dt=bf16, tagp="rA")
        ze = mktiles("ze", dt=bf16, tagp="lnA")
        ct_qt = [ctsb.tile([P, 512], bf16, name=f"ct_q{i}", tag=f"ct_q{i}",
                           bufs=1) for i in range(NCH)]
        proj_fm("ct_wq", z8, 512, ct_qt, evict_copy)
        ct_at = [trp.tile([P, 512], bf16, name=f"ct_at{i}", tag=f"at{i}",
                          bufs=2) for i in range(NCH)]
        ct_at8 = mk_at8(trp, "ct_at8")
        fin_ct = attention(ct_qt, ct_kt, ct_vts, ct_at, ct_at8, ctbias,
                           False, "ct")
        fin_ct()

        def evict_resid_r2(j, cs, ce, ps, out_tiles):
            nc.vector.scalar_tensor_tensor(out_tiles[j][:, cs:ce], ps, WINV,
                                           z[j], op0=OP.mult, op1=OP.add)
        tap("ct_at", ct_at)
        proj_fm("ct_wo", ct_at8, 512, r3, evict_resid_r2)
        tap("r3", r3)
        ln3 = layernorm(r3, ze, 2)
        ln3()
        tap("ze", ze)

        # --- stage 4: FFN (bf16 for accuracy) ---
        ffsb = ctx.enter_context(tc.tile_pool(name="ff_sb", bufs=1))
        ht = [ffsb.tile([P, 512], bf16, name=f"ff_h{i}", tag=f"ff_h{i}",
                        bufs=1) for i in range(DFF // P)]
        for jf in range(DFF // P):
            ps = psA.tile([P, 512], mybir.dt.float32, name="ff_ps",
                          tag="pps")
            for i in range(NCH):
                nc.tensor.matmul(ps, w1t[:, i * DFF + jf * P:
                                         i * DFF + (jf + 1) * P],
                                 ze[i], start=(i == 0), stop=(i == NCH - 1))
            if jf % 2 == 0:
                nc.scalar.activation(ht[jf], ps, AF.Relu)
            else:
                nc.vector.tensor_scalar_max(ht[jf], ps, 0.0)
        r4 = mktiles("r4", dt=bf16, tagp="rB")
        for j in range(NCH):
            ps = psA.tile([P, 512], mybir.dt.float32, name="ff_ps2",
                          tag="pps")
            for i in range(DFF // P):
                nc.tensor.matmul(ps, w2t[:, i * D + j * P: i * D + (j + 1) * P],
                                 ht[i], start=(i == 0),
                                 stop=(i == DFF // P - 1))
            nc.vector.tensor_add(r4[j], ps, ze[j])
        fin = [trp.tile([P, 512], mybir.dt.float32, name=f"fin{i}",
                        tag=f"fin{i}", bufs=1) for i in range(NCH)]
        ln4 = layernorm(r4, fin, 3)
        ln4()
        for j in range(NCH):
            nc.sync.dma_start(out=outT[j * P:(j + 1) * P, :], in_=fin[j])

    nc.compile()
    return nc


# ---------------------------------------------------------------------------
# host-side input preparation
# ---------------------------------------------------------------------------

def _prep_shared(inputs):
    """Cast/transform weights shared by every core."""
    sh = {}
    for n in ("sa", "cc", "ct", "ck"):
        for p in ("wq", "wk", "wv", "wo"):
            sh[f"{n}_{p}"] = np.ascontiguousarray(
                (inputs[f"{n}_{p}"].astype(F32) * WSCALE).astype(F8))
    sh["ffn_w1b"] = np.ascontiguousarray(inputs["ffn_w1"].astype(BF16))
    sh["ffn_w2b"] = np.ascontiguousarray(inputs["ffn_w2"].astype(BF16))
    gw = inputs["gate_w"].astype(F32)
    # fold the W_o projections into the gate vectors: the gate logit is
    # computed directly from the attention outputs as
    # u_cc^T cc_at + u_ck^T ck_at with u = W_o @ gate_w
    sh["gwA"] = np.ascontiguousarray(
        (inputs["cc_wo"].astype(F32) @ gw[:D] * GSCALE).astype(F8))
    sh["gwB"] = np.ascontiguousarray(
        (inputs["ck_wo"].astype(F32) @ gw[D:] * GSCALE).astype(F8))
    kl, ql = np.arange(P)[:, None], np.arange(P)[None, :]
    sh["stair"] = np.where(kl <= ql, 0.0, np.float32(-65000.0)).astype(BF16)
    sel8 = np.zeros((8, 4 * P), BF16)
    for hp in range(4):
        sel8[2 * hp, hp * P:hp * P + 64] = 1.0
        sel8[2 * hp + 1, hp * P + 64:(hp + 1) * P] = 1.0
    sh["sel8"] = sel8
    return sh


def _len_bias(L, kts, width=P):
    """[width,1] f32 additive bias for the LAST kv tile."""
    base = (kts - 1) * P
    idx = base + np.arange(width)
    return np.where(idx < L, 0.0, NEG).astype(F32)[:, None]


def _prep_core(inputs, sh, b, qh, kts_cc, kts_ct):
    KVn = 512 * (qh + 1)
    QOFF = qh * 512
    m = dict(sh)
    xT = np.ascontiguousarray(inputs["x"][b].T.astype(F32))  # [D, T]
    m["xkvT"] = np.ascontiguousarray(xT[:, :KVn].astype(F8))
    m["xqT"] = np.ascontiguousarray(
        xT[:, QOFF:QOFF + 512].astype(BF16))
    Ls = int(inputs["source_code_len"][b])
    st = np.zeros((D, kts_cc * P), F8)
    st[:, :Ls] = inputs["source_code_enc"][b, :Ls].T.astype(F8)
    m["srcT"] = st
    Lt = int(inputs["template_len"][b])
    tt = np.zeros((D, kts_ct * P), F8)
    tt[:, :Lt] = inputs["template_enc"][b, :Lt].T.astype(F8)
    m["tmplT"] = tt
    m["kwT"] = np.ascontiguousarray(inputs["keywords_enc"][b].T.astype(F8))
    m["cc_bias"] = _len_bias(Ls, kts_cc)
    m["ct_bias"] = _len_bias(Lt, kts_ct)
    m["kw_bias"] = _len_bias(int(inputs["keywords_len"][b]), 1, KW)
    return m


# ---------------------------------------------------------------------------
# concurrent multi-program PJRT runner (adapted from bass2jax.run_bass_via_pjrt)
# ---------------------------------------------------------------------------

def _run_groups(groups):
    """groups: list of (nc, core_ids, in_maps).  Dispatch all groups onto
    their own device subsets, then gather.  Returns {core_id: {name: arr}}."""
    import jax
    import numpy as _np
    from jax.sharding import Mesh, PartitionSpec
    from jax.experimental.shard_map import shard_map
    from concourse import bass2jax
    from concourse.bass2jax import (_bass_exec_p, install_neuronx_cc_hook,
                                    partition_id_tensor)

    install_neuronx_cc_hook()
    devices = jax.devices()

    def make_launch(nc, core_ids, in_maps):
        pname = (nc.partition_id_tensor.name
                 if nc.partition_id_tensor else None)
        in_names, out_names, out_avals, zero_outs = [], [], [], []
        for alloc in nc.m.functions[0].allocations:
            if not isinstance(alloc, mybir.MemoryLocationSet):
                continue
            name = alloc.memorylocations[0].name
            if alloc.kind == "ExternalInput":
                if name == pname:
                    continue
                in_names.append(name)
            elif alloc.kind == "ExternalOutput":
                shape = tuple(alloc.tensor_shape)
                dtype = mybir.dt.np(alloc.dtype)
                out_names.append(name)
                out_avals.append(jax.core.ShapedArray(shape, dtype))
                zero_outs.append(_np.zeros(shape, dtype))
        n_params, n_outs = len(in_names), len(out_avals)
        all_in_names = in_names + out_names
        if pname is not None:
            all_in_names = all_in_names + [pname]

        def _body(*args):
            operands = list(args)
            if pname is not None:
                operands.append(partition_id_tensor())
            outs = _bass_exec_p.bind(
                *operands, out_avals=tuple(out_avals),
                in_names=tuple(all_in_names), out_names=tuple(out_names),
                lowering_input_output_aliases=(),
                sim_require_finite=False, sim_require_nnan=False, nc=nc)
            return tuple(outs)

        donate = tuple(range(n_params, n_params + n_outs))
        devs = [devices[c] for c in core_ids]
        if len(core_ids) == 1:
            fn = jax.jit(_body, donate_argnums=donate, keep_unused=True,
                         device=devs[0])
            args = [in_maps[0][nm] for nm in in_names] + list(zero_outs)
            out_arrs = fn(*args)
            return out_names, out_avals, out_arrs, None
        mesh = Mesh(_np.asarray(devs), ("core",))
        in_specs = (PartitionSpec("core"),) * (n_params + n_outs)
        out_specs = (PartitionSpec("core"),) * n_outs
        fn = jax.jit(shard_map(_body, mesh=mesh, in_specs=in_specs,
                               out_specs=out_specs, check_rep=False),
                     donate_argnums=donate, keep_unused=True)
        cat = [_np.concatenate([_np.asarray(m[nm]) for m in in_maps], axis=0)
               for nm in in_names]
        catz = [_np.zeros((len(core_ids) * z.shape[0], *z.shape[1:]), z.dtype)
                for z in zero_outs]
        out_arrs = fn(*cat, *catz)
        return out_names, out_avals, out_arrs, len(core_ids)

    last_err = None
    for _attempt in range(3):
        try:
            launched = []
            for nc, core_ids, in_maps in groups:
                launched.append((core_ids, make_launch(nc, core_ids, in_maps)))
            results = {}
            for core_ids, (out_names, out_avals, out_arrs, ncores) in launched:
                if ncores is None:
                    results[core_ids[0]] = {nm: _np.asarray(out_arrs[i])
                                            for i, nm in enumerate(out_names)}
                else:
                    for ci, c in enumerate(core_ids):
                        results[c] = {
                            nm: _np.asarray(out_arrs[i]).reshape(
                                ncores, *out_avals[i].shape)[ci]
                            for i, nm in enumerate(out_names)}
            return results
        except Exception as e:  # transient NRT device errors: retry
            last_err = e
            import time as _time
            _time.sleep(2.0)
    raise last_err


_PROGRAM_CACHE = {}
_CACHE_LOCK = threading.Lock()


def _get_program(key):
    with _CACHE_LOCK:
        if key in _PROGRAM_CACHE:
            return _PROGRAM_CACHE[key]
    qh, kts_cc, kts_ct, gate_b, aff = key
    nc = build_program(qh, kts_cc, kts_ct, gate_b=gate_b, apply_affine=aff)
    with _CACHE_LOCK:
        _PROGRAM_CACHE[key] = nc
    return nc


# ---------------------------------------------------------------------------
# entry point
# ---------------------------------------------------------------------------

def kernel(**inputs):
    inputs = {k: np.asarray(v) for k, v in inputs.items()}
    gate_b = float(inputs["gate_b"].reshape(-1)[0])
    aff = not all(
        np.all(inputs[f"ln{j}_g"] == 1.0) and np.all(inputs[f"ln{j}_b"] == 0.0)
        for j in range(1, 5))
    affine_arr = None
    if aff:
        affine_arr = np.zeros((P, NCH * 8), F32)
        for ln in range(4):
            g = inputs[f"ln{ln + 1}_g"].astype(F32).reshape(NCH, P).T
            bb = inputs[f"ln{ln + 1}_b"].astype(F32).reshape(NCH, P).T
            affine_arr[:, ln * 2 * NCH: ln * 2 * NCH + NCH] = g
            affine_arr[:, ln * 2 * NCH + NCH: (ln + 1) * 2 * NCH] = bb

    sh = _prep_shared(inputs)
    # core -> (program key, in_map)
    core_keys, core_maps = [], []
    for c in range(8):
        b, qh = c // 2, c % 2
        kts_cc = max(1, -(-int(inputs["source_code_len"][b]) // P))
        kts_ct = max(1, -(-int(inputs["template_len"][b]) // P))
        key = (qh, kts_cc, kts_ct, gate_b, aff)
        m = _prep_core(inputs, sh, b, qh, kts_cc, kts_ct)
        if aff:
            m["ln_affine"] = affine_arr
        core_keys.append(key)
        core_maps.append(m)

    # build distinct programs (parallel threads: walrus compile is subprocess)
    distinct = sorted(set(core_keys))
    threads = [threading.Thread(target=_get_program, args=(k,))
               for k in distinct]
    for t in threads:
        t.start()
    for t in threads:
        t.join()

    groups = []
    for key in distinct:
        cores = [c for c in range(8) if core_keys[c] == key]
        groups.append((_get_program(key), cores, [core_maps[c] for c in cores]))

    results = _run_groups(groups)

    out = np.empty((B, T, D), np.float32)
    for c in range(8):
        b, qh = c // 2, c % 2
        out[b, qh * 512:(qh + 1) * 512, :] = results[c]["outT"].T
    return out



# revision 28
# speedup vs baseline: 1.0781x; 1.0362x over previous
"""Trainium2 Bass kernel for nn_DecoderBlockWithKeywords.

Decoder block: causal self-attn + gated (source-code / keywords) cross-attn
+ template cross-attn + FFN, with 4 LayerNorms.  B=4, T=1024, D=512, H=8,
dh=64, DFF=2048.

Sharding: pure data-parallel over (batch, query-half) -> 8 NeuronCores, no
collectives.  Each core holds all weights (fp16) and computes 512 query
tokens of one batch element.

Layout strategy: every activation lives feature-major (X^T: [D on
partitions, tokens on free]).  Host pre-transposes/casts inputs.  Q/K
projections are weight-stationary (out feature-major); V is produced
token-major via activation-stationary matmuls so the attention AV matmul
needs no transposes at all.  Scores are computed as S^T = K_h Q_h^T
([kv, q]); softmax runs without max-subtraction (logits are O(1); masked
lanes get -1e6 bias fused into the ACT exp).  Softmax denominators come
from a ones-column appended to V inside the same AV matmul; per-column
scales (softmax 1/n, LN mean/rstd, gate g) are broadcast across partitions
with a PE ones-outer-product into a free PSUM bank and applied by one DVE
op reading that PSUM operand.  LayerNorm is done feature-major: column sums
via PE ones-matmuls, rstd = exp(-0.5*ln(v)) on ACT (single activation-table
set, zero table switches).  Residuals follow the reference post-LN chaining
(z = LN(y + y2), z_end = LN(z + z2), out = LN(z_end + ff)).

Programs are specialized at build time to the actual kv lengths (read from
the int32 length inputs), so masked kv tiles are skipped entirely; up to 8
distinct programs (4 batches x even/odd query half) are compiled and
launched concurrently on disjoint device subsets.
"""

import os
import sys
import threading

import numpy as np

for _p in ("/opt/trn_rl_repo", "/root/.axon_site"):
    if os.path.isdir(_p) and _p not in sys.path:
        sys.path.append(_p)

import ml_dtypes
from contextlib import ExitStack

import concourse.bass as bass
import concourse.mybir as mybir
from concourse import bacc
from concourse.tile import TileContext

BF16 = np.float16
F32 = np.float32
F8 = ml_dtypes.float8_e4m3
WSCALE = 16.0      # weights stored as w*16 in fp8 (values ~N(0,0.02))
WINV = 1.0 / WSCALE
GSCALE = 64.0      # gate u-vectors stored as u*64 in fp8
NEG = -1000000.0
B, T, S, TM, KW, D, H, DFF = 4, 1024, 1024, 512, 64, 512, 8, 2048
DH = D // H  # 64
P = 128
NCH = D // P  # 4 feature chunks
AF = mybir.ActivationFunctionType
OP = mybir.AluOpType


# ---------------------------------------------------------------------------
# program builder
# ---------------------------------------------------------------------------

def build_program(qh, kts_cc, kts_ct, gate_b=0.0, apply_affine=False, debug=False):
    """Build one core's Bass program.

    qh: 0/1 query half.  kts_cc/kts_ct: number of 128-wide kv tiles for the
    source-code / template cross attentions (specialized to actual length).
    """
    f32, bf16 = mybir.dt.float32, mybir.dt.float16
    f8 = mybir.dt.float8e4
    DR = mybir.MatmulPerfMode.DoubleRow
    # qh selects the q-token split that EQUALIZES causal work:
    #   variant 0 (outer): q in [0,256) u [768,1024), kv range 1024
    #   variant 1 (inner): q in [256,768), kv range 768
    # cspec: per kv tile (live col start c0, stair diagonal block?)
    if qh == 0:
        KV = 1024
        CSPEC = [(0, True), (128, True), (256, False), (256, False),
                 (256, False), (256, False), (256, True), (384, True)]
    else:
        KV = 768
        CSPEC = [(0, False), (0, False), (0, True), (128, True),
                 (256, True), (384, True)]

    nc = bacc.Bacc("TRN2", target_bir_lowering=False, debug=False)

    def din(name, shape, dt=bf16):
        return nc.dram_tensor(name, shape, dt, kind="ExternalInput").ap()

    xkvT = din("xkvT", [D, KV], f8)
    xq8T = din("xq8T", [D, 512], f8)  # fp8 x at this core's q columns
    xqT = din("xqT", [D, 512])       # bf16 x for the residual add
    srcT = din("srcT", [D, kts_cc * P], f8)
    tmplT = din("tmplT", [D, kts_ct * P], f8)
    kwT = din("kwT", [D, KW], f8)
    wnames = [f"{n}_{p}" for n in ("sa", "cc", "ct", "ck")
              for p in ("wq", "wk", "wv", "wo")]
    wd = {n: din(n, [D, D], f8) for n in wnames}
    w1bd = din("ffn_w1b", [D, DFF])
    w2bd = din("ffn_w2b", [DFF, D])
    gwA = din("gwA", [D, 1], f8)
    gwB = din("gwB", [D, 1], f8)
    staird = din("stair", [P, P])
    sel8_d = din("sel8", [H, NCH * P])
    ccbias_d = din("cc_bias", [P, 1], f32)
    ctbias_d = din("ct_bias", [P, 1], f32)
    kwbias_d = din("kw_bias", [KW, 1], f32)
    affine_d = din("ln_affine", [P, NCH * 8], f32) if apply_affine else None
    outT = nc.dram_tensor("outT", [D, 512], bf16, kind="ExternalOutput").ap()
    dbg_outs = {}

    def mkdbg(nm, shape):
        if nm not in dbg_outs:
            dbg_outs[nm] = nc.dram_tensor(f"dbg_{nm}", shape, f32,
                                          kind="ExternalOutput").ap()
        return dbg_outs[nm]

    with TileContext(nc, pool_alloc_mode="queue") as tc, ExitStack() as ctx:
        # Pre-place one ACT table covering Exp+Ln+Copy+Square+Relu: without
        # this the compiler's greedy chooser ping-pongs exp_and_others <->
        # natural_log (~19 loads x 1.3us on the critical path).
        from concourse.hw_specs import get_activation_tables
        _tabs = list(get_activation_tables(nc.m.arch).keys())
        nc.scalar.add_instruction(mybir.InstLoadActFuncSet(
            act_func_set_id=_tabs.index("natural_log_exp_and_others"),
            name=nc.get_next_instruction_name(),
            engine=mybir.EngineType.Activation))
        pers = ctx.enter_context(tc.tile_pool(name="pers", bufs=1))
        # ---- persistent small constants -------------------------------
        # (const DMAs are emitted by emit_consts() AFTER the stage-1
        # critical weight/activation loads so they don't delay the first
        # projection)
        stair = pers.tile([P, P], bf16, name="stair_t")
        ccbias = pers.tile([P, 1], f32, name="ccbias_t")
        ctbias = pers.tile([P, 1], f32, name="ctbias_t")
        kwbias = pers.tile([KW, 1], f32, name="kwbias_t")
        gwa_t = pers.tile([P, NCH], f8, name="gwa_t")
        gwb_t = pers.tile([P, NCH], f8, name="gwb_t")
        sel8 = pers.tile([H, NCH * P], bf16, name="sel8_t")
        affine = None
        if apply_affine:
            affine = pers.tile([P, NCH * 8], f32, name="affine_t")

        def emit_consts():
            nc.sync.dma_start(out=stair, in_=staird)
            nc.sync.dma_start(out=ccbias, in_=ccbias_d)
            nc.sync.dma_start(out=ctbias, in_=ctbias_d)
            nc.sync.dma_start(out=kwbias, in_=kwbias_d)
            nc.sync.dma_start(out=gwa_t,
                              in_=gwA.rearrange("(i p) o -> p i o", p=P))
            nc.sync.dma_start(out=gwb_t,
                              in_=gwB.rearrange("(i p) o -> p i o", p=P))
            nc.sync.dma_start(out=sel8, in_=sel8_d)
            if apply_affine:
                nc.sync.dma_start(out=affine, in_=affine_d)

        ones_f = pers.tile([P, 1], f32, name="ones_f")
        nc.vector.memset(ones_f, 1.0)
        ones_b = pers.tile([P, 1], bf16, name="ones_b")
        nc.vector.memset(ones_b, 1.0)
        ones_row = pers.tile([1, P], bf16, name="ones_row")
        nc.vector.memset(ones_row, 1.0)
        eps_t = pers.tile([1, 1], f32, name="eps_t")
        nc.vector.memset(eps_t, 1e-5)
        gb_t = pers.tile([1, 1], f32, name="gb_t")
        nc.vector.memset(gb_t, -float(gate_b))

        def tap(nm, tiles):
            if not debug:
                return
            cols = tiles[0].shape[-1]
            d = mkdbg(nm, [len(tiles) * P, cols])
            for i, t in enumerate(tiles):
                rows = t.shape[0]
                nc.gpsimd.dma_start(out=d[i * P:i * P + rows, :], in_=t)

        # ---- global shared pools --------------------------------------
        # residual/LN-out tiles, reused across stages via shared tags
        rpool = ctx.enter_context(tc.tile_pool(name="rpool", bufs=1))

        def mktiles(nm, cols=512, dt=f32, n=NCH, tagp=None):
            tagp = tagp or nm
            return [rpool.tile([P, cols], dt, name=f"{nm}{i}", tag=f"{tagp}{i}",
                               bufs=1) for i in range(n)]

        # small 1/8-partition tiles + broadcast tiles, shared by all stages
        smallp = ctx.enter_context(tc.tile_pool(name="smallp", bufs=1))
        # transient [128, *] tiles (exp outputs, LN scratch, gate scratch)
        trp = ctx.enter_context(tc.tile_pool(name="trp", bufs=1))
        # PSUM: pps = projection/V accumulators; x_ps = paired scores
        # (2 banks each); x_po = AV out + LN stats + gate
        psA = ctx.enter_context(tc.tile_pool(name="psA", bufs=2, space="PSUM"))
        psB = ctx.enter_context(tc.tile_pool(name="psB", bufs=2, space="PSUM"))

        def load_w(pool, names):
            # fp8 weights in DoubleRow-paired layout: free dims
            # (ipair, two, out-col)
            for n in names:
                wt[n] = pool.tile([P, NCH * D], f8, name=f"{n}_t",
                                  tag=f"{n}_t", bufs=1)
                nc.sync.dma_start(
                    out=wt[n].rearrange("p (ip two n) -> p ip two n",
                                        ip=2, two=2),
                    in_=wd[n].rearrange("(ip two p) n -> p ip two n",
                                        p=P, two=2))
        wt = {}

        def w_lhsT(n, ip, j):
            # [128, 2, 128] stationary pair for DoubleRow
            return wt[n].rearrange("p (ip two n) -> p ip two n",
                                   ip=2, two=2)[:, ip, :, j * P:(j + 1) * P]

        def w_rhs(n, ip, cols=D):
            # [128, 2, cols] moving pair for DoubleRow
            return wt[n].rearrange("p (ip two n) -> p ip two n",
                                   ip=2, two=2)[:, ip, :, 0:cols]

        def load_act(pool, nm, dram_ap, cols):
            # fp8 activations as 2 chunk-paired tiles [P, 2, cols]
            tiles = []
            for pi in range(2):
                t = pool.tile([P, 2 * cols], f8, name=f"{nm}{pi}",
                              tag=f"{nm}{pi}", bufs=1)
                nc.sync.dma_start(
                    out=t.rearrange("p (two n) -> p two n", two=2),
                    in_=dram_ap[pi * 2 * P:(pi + 1) * 2 * P, :].rearrange(
                        "(two p) n -> p two n", p=P))
                tiles.append(t)
            return tiles

        def pair_view(t):
            return t.rearrange("p (two n) -> p two n", two=2)

        # ----------------------------------------------------------------
        # helpers
        # ----------------------------------------------------------------
        def proj_fm(wn, rhs_pairs, ncols, out_tiles, evict, coff=0):
            # rhs_pairs: 2 chunk-paired fp8 tiles [P, 2, >=coff+ncols]
            ntt = (ncols + 511) // 512
            for j in range(NCH):
                for t in range(ntt):
                    cs = t * 512
                    ce = min(ncols, cs + 512)
                    ps = psA.tile([P, ce - cs], mybir.dt.float32,
                                  name="proj_ps", tag="pps")
                    for ip in range(2):
                        nc.tensor.matmul(
                            ps, w_lhsT(wn, ip, j),
                            pair_view(rhs_pairs[ip])[:, :,
                                                     coff + cs:coff + ce],
                            start=(ip == 0), stop=(ip == 1),
                            perf_mode=DR)
                    evict(j, cs, ce, ps, out_tiles)

        def evict_copy(j, cs, ce, ps, out_tiles):
            nc.vector.tensor_scalar_mul(out_tiles[j][:, cs:ce], ps, WINV)

        def proj_v(enc_pairs, wn, nkv, vt_list, vpool, ktag):
            # produces kv-tile-PAIRED fp8 V tiles [rows, nsub, H*(DH+1)]
            # (nsub=2 except a trailing odd tile) for DoubleRow AV matmuls
            nch_tok = (nkv + P - 1) // P
            for m in range(nch_tok):
                rows = min(P, nkv - m * P)
                ps = psA.tile([rows, D], mybir.dt.float32,
                              name="v_ps", tag="pps")
                for ip in range(2):
                    nc.tensor.matmul(
                        ps, pair_view(enc_pairs[ip])[:, :, m * P:m * P + rows],
                        w_rhs(wn, ip),
                        start=(ip == 0), stop=(ip == 1), perf_mode=DR)
                if m % 2 == 0:
                    nsub = 2 if m + 1 < nch_tok else 1
                    vtp = vpool.tile([rows, nsub * H * (DH + 2)], f8,
                                     name=f"{ktag}_v{m // 2}",
                                     tag=f"{ktag}_v{m // 2}", bufs=1)
                    vt_list.append((vtp, nsub, rows))
                vtp, nsub, _ = vt_list[-1]
                v4 = vtp.rearrange("p (two g c) -> p two g c",
                                   two=nsub, c=DH + 2)
                nc.any.tensor_scalar_mul(
                    v4[:, m % 2, :, 0:DH],
                    ps.rearrange("p (g c) -> p g c", c=DH), WINV)
                nc.vector.memset(v4[:, m % 2, :, DH:DH + 2], 1.0)

        def attention(qt, kt, vt_pairs, out_tiles, out8, bias_tile, causal,
                      ktag):
            """Multi-head attention, head pairs share one [rows,1024]
            scores psum + one merged exp (fp8 out, kv-pair planes).  AV is
            a DoubleRow fp8 matmul over kv-tile pairs.  Causal tiles only
            touch live query columns.  Returns a finish() closure that
            emits the normalizer broadcasts + final muls, so callers can
            interleave independent PE work with the 1/n scalar chain."""
            nkt = sum(ns for _, ns, _ in vt_pairs)
            npairs = len(vt_pairs)
            nmat = smallp.tile([H, 512], mybir.dt.float32,
                               name=f"{ktag}_nmat", tag="nmat", bufs=1)
            for hp in range(H // 2):
                po = []
                for s in range(2):
                    po.append(psB.tile([DH + 2, 512], mybir.dt.float32,
                                       name=f"{ktag}_po{s}", tag="x_po"))
                kt_base = 0
                for mp, (vtp, nsub, vrows) in enumerate(vt_pairs):
                    specs = [CSPEC[kt_base + u] if causal else (0, False)
                             for u in range(nsub)]
                    c0s = [sp[0] for sp in specs]
                    c0p = c0s[0]
                    pp = trp.tile([vrows, nsub * 1024], f8,
                                  name=f"{ktag}_pt", tag="pt", bufs=3)
                    pp3 = pp.rearrange("p (two n) -> p two n", two=nsub)
                    for u in range(nsub):
                        kt_i = kt_base + u
                        c0, st = specs[u]
                        ps2 = psB.tile([vrows, 1024], mybir.dt.float32,
                                       name=f"{ktag}_ps", tag="x_ps")
                        for s in range(2):
                            ro = s * DH
                            o = s * 512
                            nc.tensor.matmul(
                                ps2[:, o + c0:o + 512],
                                kt[hp][ro:ro + DH,
                                       kt_i * P:kt_i * P + vrows],
                                qt[hp][ro:ro + DH, c0:512],
                                start=True, stop=True)
                        if causal and (st or c0 > 0):
                            for s in range(2):
                                o = s * 512
                                if st:
                                    nc.vector.tensor_add(
                                        ps2[:, o + c0:o + c0 + P],
                                        ps2[:, o + c0:o + c0 + P], stair)
                                if c0 > c0p:
                                    nc.vector.memset(
                                        pp3[:, u, o + c0p:o + c0], 0.0)
                            # one strided exp covers both heads' live range
                            nc.scalar.activation(
                                pp.rearrange(
                                    "p (two s n) -> p two s n",
                                    two=nsub, s=2)[:, u, :, c0:512],
                                ps2.rearrange("p (s n) -> p s n",
                                              s=2)[:, :, c0:512],
                                AF.Exp, scale=0.125)
                        else:
                            bias = 0.0
                            if bias_tile is not None and kt_i == nkt - 1:
                                bias = bias_tile[:vrows, :]
                            nc.scalar.activation(pp3[:, u, :], ps2, AF.Exp,
                                                 bias=bias, scale=0.125)
                    v4 = vtp.rearrange("p (two g c) -> p two g c",
                                       two=nsub, c=DH + 2)
                    for s in range(2):
                        h = 2 * hp + s
                        o = s * 512
                        if nsub == 2:
                            nc.tensor.matmul(
                                po[s][:, c0p:512], v4[:, :, h, :],
                                pp3[:, :, o + c0p:o + 512],
                                start=(mp == 0), stop=(mp == npairs - 1),
                                skip_group_check=True, perf_mode=DR)
                        else:
                            nc.tensor.matmul(
                                po[s][:, c0p:512], v4[:, 0, h, :],
                                pp3[:, 0, o + c0p:o + 512],
                                start=(mp == 0), stop=(mp == npairs - 1),
                                skip_group_check=True)
                    kt_base += nsub
                for s in range(2):
                    ro = s * DH
                    h = 2 * hp + s
                    nscr = smallp.tile([1, 512], mybir.dt.float32,
                                       name=f"{ktag}_nscr{h}", tag="nscr",
                                       bufs=2)
                    nc.any.tensor_copy(nscr, po[s][DH:DH + 1, :])
                    nc.sync.dma_start(out=nmat[h:h + 1, :], in_=nscr)
                    nc.any.tensor_copy(out_tiles[hp][ro:ro + DH, :],
                                       po[s][0:DH, :])
            lnn = smallp.tile([H, 512], mybir.dt.float32,
                              name=f"{ktag}_lnn", tag="lnn", bufs=1)
            nc.scalar.activation(lnn, nmat, AF.Ln)
            ninv8 = smallp.tile([H, 512], bf16,
                                name=f"{ktag}_ninv8", tag=f"ninv8_{ktag}",
                                bufs=1)
            nc.scalar.activation(ninv8, lnn, AF.Exp, scale=-1.0)

            def finish():
                for hp in range(H // 2):
                    nb = psA.tile([P, 512], mybir.dt.float32,
                                  name=f"{ktag}_nb", tag="pps")
                    nc.tensor.matmul(nb, sel8[:, hp * P:(hp + 1) * P],
                                     ninv8, start=True, stop=True)
                    nc.vector.tensor_mul(
                        pair_view(out8[hp // 2])[:, hp % 2, :],
                        out_tiles[hp], nb)
            return finish

        def layernorm(r_tiles, out_tiles, ln_idx, out8=None):
            sq = [trp.tile([P, 512], bf16, name=f"ln{ln_idx}_sq", tag="ln_sq",
                           bufs=4) for _ in range(NCH)]
            for j in range(NCH):
                nc.scalar.activation(sq[j], r_tiles[j], AF.Square)
            ps_s = psB.tile([1, 512], mybir.dt.float32,
                            name="ln_ps_s", tag="x_po")
            ps_q = psB.tile([1, 512], mybir.dt.float32,
                            name="ln_ps_q", tag="x_po")
            for j in range(NCH):
                nc.tensor.matmul(ps_s, ones_b, r_tiles[j],
                                 start=(j == 0), stop=(j == NCH - 1))
            # mean/mean^2 depend only on ps_s: emit before the ps_q sums so
            # only var->rstd remains on the chain after the last sum
            mean16 = smallp.tile([1, 512], bf16,
                                 name="ln_mean16", tag="ln_stat", bufs=3)
            nc.vector.tensor_scalar_mul(mean16, ps_s, 1.0 / D)
            msq = smallp.tile([1, 512], mybir.dt.float32,
                              name="ln_msq", tag="ln_stat", bufs=3)
            nc.vector.tensor_mul(msq, mean16, mean16)
            for j in range(NCH):
                nc.tensor.matmul(ps_q, ones_b, sq[j],
                                 start=(j == 0), stop=(j == NCH - 1))
            var = smallp.tile([1, 512], mybir.dt.float32,
                              name="ln_var", tag="ln_stat", bufs=3)
            nc.vector.scalar_tensor_tensor(var, ps_q, 1.0 / D, msq,
                                           op0=OP.mult, op1=OP.subtract)
            lnv = smallp.tile([1, 512], mybir.dt.float32,
                              name="ln_lnv", tag="ln_stat", bufs=3)
            nc.scalar.activation(lnv, var, AF.Ln, bias=eps_t[:, :])
            rstd = smallp.tile([1, 512], bf16,
                               name="ln_rstd", tag="ln_stat", bufs=3)
            nc.scalar.activation(rstd, lnv, AF.Exp, scale=-0.5)

            def apply():
                meanb = psB.tile([P, 512], mybir.dt.float32,
                                 name="ln_meanb", tag="x_po")
                nc.tensor.matmul(meanb, ones_row, mean16,
                                 start=True, stop=True)
                rstdb = psB.tile([P, 512], mybir.dt.float32,
                                 name="ln_rstdb", tag="x_po")
                nc.tensor.matmul(rstdb, ones_row, rstd,
                                 start=True, stop=True)
                for j in range(NCH):
                    tmp = trp.tile([P, 512], bf16,
                                   name="ln_tmp", tag="ln_tmp", bufs=2)
                    nc.vector.tensor_sub(tmp, r_tiles[j], meanb)
                    nc.vector.tensor_mul(out_tiles[j], tmp, rstdb)
                    if apply_affine:
                        g = affine[:, ln_idx * 2 * NCH + j:
                                   ln_idx * 2 * NCH + j + 1]
                        b = affine[:, ln_idx * 2 * NCH + NCH + j:
                                   ln_idx * 2 * NCH + NCH + j + 1]
                        nc.vector.tensor_scalar(out_tiles[j], out_tiles[j],
                                                g, b, op0=OP.mult,
                                                op1=OP.add)
                    if out8 is not None:
                        nc.scalar.activation(
                            pair_view(out8[j // 2])[:, j % 2, :],
                            out_tiles[j], AF.Copy)
            return apply

        # ================================================================
        # emission (ordered for cross-stage overlap)
        # ================================================================
        r1 = mktiles("r1", dt=bf16, tagp="rA")
        y = mktiles("y", dt=bf16, tagp="lnA")
        r2 = mktiles("r2", dt=bf16, tagp="rB")
        z = mktiles("z", dt=bf16, tagp="lnB")
        r3 = None  # allocated after r1 dies
        ze = None

        ccsb = ctx.enter_context(tc.tile_pool(name="cc_sb", bufs=1))
        sasb_cm = tc.tile_pool(name="sa_sb", bufs=1)
        sasb = sasb_cm.__enter__()

        def mk_at8(pool, nm):
            return [pool.tile([P, 1024], f8, name=f"{nm}{i}",
                              tag=f"at8_{i}", bufs=2) for i in range(2)]

        # --- stage 1: self attention ---
        load_w(sasb, ["sa_wk"])
        xkv = load_act(sasb, "xkv", xkvT, KV)
        xq8 = load_act(sasb, "xq8", xq8T, 512)
        xq = []
        for i in range(NCH):
            t = sasb.tile([P, 512], bf16, name=f"xq{i}", tag=f"xq{i}",
                          bufs=1)
            nc.sync.dma_start(out=t, in_=xqT[i * P:(i + 1) * P, :])
            xq.append(t)
        load_w(sasb, ["sa_wv", "sa_wq", "sa_wo"])
        emit_consts()
        qt = [sasb.tile([P, 512], bf16, name=f"sa_q{i}", tag=f"sa_q{i}",
                        bufs=1) for i in range(NCH)]
        ktl = [sasb.tile([P, KV], bf16, name=f"sa_k{i}", tag=f"sa_k{i}",
                         bufs=1) for i in range(NCH)]
        proj_fm("sa_wk", xkv, KV, ktl, evict_copy)
        vts = []
        proj_v(xkv, "sa_wv", KV, vts, sasb, "sa")
        proj_fm("sa_wq", xq8, 512, qt, evict_copy)
        at = [trp.tile([P, 512], bf16, name=f"sa_at{i}", tag=f"at{i}",
                       bufs=2) for i in range(NCH)]
        at8 = mk_at8(trp, "sa_at8")
        fin_sa = attention(qt, ktl, vts, at, at8, None, True, "sa")

        # hoist: cc/ck K+V projections are independent of LN1; the cc_wk
        # projection also fills the sa-normalizer scalar chain
        load_w(ccsb, ["cc_wk", "cc_wv", "ck_wk", "ck_wv",
                      "cc_wq", "ck_wq", "cc_wo", "ck_wo"])
        srcl = load_act(ccsb, "src", srcT, kts_cc * P)
        kwe = load_act(ccsb, "kw", kwT, KW)
        cc_kt = [ccsb.tile([P, kts_cc * P], bf16, name=f"cc_k{i}",
                           tag=f"cc_k{i}", bufs=1) for i in range(NCH)]
        proj_fm("cc_wk", srcl, kts_cc * P, cc_kt, evict_copy)
        fin_sa()
        cc_vts = []
        proj_v(srcl, "cc_wv", kts_cc * P, cc_vts, ccsb, "cc")

        def evict_resid_x(j, cs, ce, ps, out_tiles):
            nc.vector.scalar_tensor_tensor(out_tiles[j][:, cs:ce], ps, WINV,
                                           xq[j], op0=OP.mult, op1=OP.add)
        tap("sa_at", at)
        proj_fm("sa_wo", at8, 512, r1, evict_resid_x)
        tap("r1", r1)
        ck_kt = [ccsb.tile([P, KW], bf16, name=f"ck_k{i}", tag=f"ck_k{i}",
                           bufs=1) for i in range(NCH)]
        proj_fm("ck_wk", kwe, KW, ck_kt, evict_copy)
        y8 = [rpool.tile([P, 1024], f8, name=f"y8_{i}", tag=f"lnA8_{i}",
                         bufs=1) for i in range(2)]
        ln1 = layernorm(r1, y, 0, out8=y8)
        ck_vts = []
        proj_v(kwe, "ck_wv", KW, ck_vts, ccsb, "ck")
        ln1()
        tap("y", y)
        sasb_cm.__exit__(None, None, None)

        # --- stage 2: cc + ck cross attention + gate ---
        cc_qt = [ccsb.tile([P, 512], bf16, name=f"cc_q{i}", tag=f"cc_q{i}",
                           bufs=1) for i in range(NCH)]
        proj_fm("cc_wq", y8, 512, cc_qt, evict_copy)
        cc_at = [trp.tile([P, 512], bf16, name=f"cc_at{i}", tag=f"at{i}",
                          bufs=2) for i in range(NCH)]
        cc_at8 = mk_at8(trp, "cc_at8")
        fin_cc = attention(cc_qt, cc_kt, cc_vts, cc_at, cc_at8, ccbias,
                           False, "cc")
        # ck q-projection fills cc's normalize tail
        ck_qt = [ccsb.tile([P, 512], bf16, name=f"ck_q{i}", tag=f"ck_q{i}",
                           bufs=1) for i in range(NCH)]
        proj_fm("ck_wq", y8, 512, ck_qt, evict_copy)
        fin_cc()
        ck_at = [trp.tile([P, 512], bf16, name=f"ck_at{i}", tag=f"ckat{i}",
                          bufs=1) for i in range(NCH)]
        ck_at8 = [trp.tile([P, 1024], f8, name=f"ck_at8_{i}",
                           tag=f"ckat8_{i}", bufs=1) for i in range(2)]
        fin_ck = attention(ck_qt, ck_kt, ck_vts, ck_at, ck_at8, kwbias,
                           False, "ck")
        # ct weight/act loads + K projection fill ck's normalize tail
        ctsb = ctx.enter_context(tc.tile_pool(name="tail_sb", bufs=1))
        load_w(ctsb, ["ct_wk", "ct_wv", "ct_wq", "ct_wo"])
        tmpl = load_act(ctsb, "tmpl", tmplT, kts_ct * P)
        ct_kt = [ctsb.tile([P, kts_ct * P], bf16, name=f"ct_k{i}",
                           tag=f"ct_k{i}", bufs=1) for i in range(NCH)]
        proj_fm("ct_wk", tmpl, kts_ct * P, ct_kt, evict_copy)
        fin_ck()

        # --- gate logits straight off the attention outputs (gwa_t/gwb_t
        # hold the host-fused u = W_o @ gate_w vectors), so the sigmoid
        # chain overlaps the W_o projections ---
        ps_g = psB.tile([1, 512], mybir.dt.float32, name="gate_ps",
                        tag="x_po")
        for i in range(NCH):
            nc.tensor.matmul(ps_g, gwa_t[:, i:i + 1],
                             pair_view(cc_at8[i // 2])[:, i % 2, :],
                             start=(i == 0), stop=False)
        for i in range(NCH):
            nc.tensor.matmul(ps_g, gwb_t[:, i:i + 1],
                             pair_view(ck_at8[i // 2])[:, i % 2, :],
                             start=False, stop=(i == NCH - 1))
        ge = smallp.tile([1, 512], mybir.dt.float32, name="gate_e",
                         tag="gate_edg", bufs=3)
        nc.scalar.activation(ge, ps_g, AF.Exp, scale=-1.0 / GSCALE,
                             bias=gb_t[:, :])
        gl2 = smallp.tile([1, 512], mybir.dt.float32, name="gate_lnd",
                          tag="gate_edg", bufs=3)
        nc.scalar.activation(gl2, ge, AF.Ln, bias=1.0)
        gg = smallp.tile([1, 512], bf16, name="gate_g",
                         tag="gate_edg", bufs=3)
        nc.scalar.activation(gg, gl2, AF.Exp, scale=-1.0)
        # first ct V-projection tile fills the gate sigmoid chain; the
        # rest interleave with the vector-bound blend loop below
        ct_vts = []
        ct_v_nch = (kts_ct * P + P - 1) // P

        def ct_v_tile(m):
            rows = min(P, kts_ct * P - m * P)
            ps = psA.tile([rows, D], mybir.dt.float32, name="v_ps",
                          tag="pps")
            for ip in range(2):
                nc.tensor.matmul(
                    ps, pair_view(tmpl[ip])[:, :, m * P:m * P + rows],
                    w_rhs("ct_wv", ip),
                    start=(ip == 0), stop=(ip == 1), perf_mode=DR)
            if m % 2 == 0:
                nsub = 2 if m + 1 < ct_v_nch else 1
                vtp = ctsb.tile([rows, nsub * H * (DH + 2)], f8,
                                name=f"ct_v{m // 2}", tag=f"ct_v{m // 2}",
                                bufs=1)
                ct_vts.append((vtp, nsub, rows))
            vtp, nsub, _ = ct_vts[-1]
            v4_ = vtp.rearrange("p (two g c) -> p two g c",
                                two=nsub, c=DH + 2)
            nc.any.tensor_scalar_mul(
                v4_[:, m % 2, :, 0:DH],
                ps.rearrange("p (g c) -> p g c", c=DH), WINV)
            nc.vector.memset(v4_[:, m % 2, :, DH:DH + 2], 1.0)

        ct_v_tile(0)
        ggb = psB.tile([P, 512], mybir.dt.float32, name="gate_gb",
                       tag="x_po")
        nc.tensor.matmul(ggb, ones_row, gg, start=True, stop=True)
        # fused blended W_o projections: r2 = y + y2k + g*(y2c - y2k)
        for j in range(NCH):
            if j + 1 < ct_v_nch:
                ct_v_tile(j + 1)
            psc = psA.tile([P, 512], mybir.dt.float32, name="wo_psc",
                           tag="pps")
            for ip in range(2):
                nc.tensor.matmul(psc, w_lhsT("cc_wo", ip, j),
                                 pair_view(cc_at8[ip]),
                                 start=(ip == 0), stop=(ip == 1),
                                 perf_mode=DR)
            psk = psA.tile([P, 512], mybir.dt.float32, name="wo_psk",
                           tag="pps")
            for ip in range(2):
                nc.tensor.matmul(psk, w_lhsT("ck_wo", ip, j),
                                 pair_view(ck_at8[ip]),
                                 start=(ip == 0), stop=(ip == 1),
                                 perf_mode=DR)
            y2ks = trp.tile([P, 512], bf16, name="gate_y2k", tag="gate_y2k",
                            bufs=2)
            nc.scalar.activation(y2ks, psk, AF.Copy, scale=WINV)
            dt_ = trp.tile([P, 512], bf16, name="gate_dt", tag="gate_dt",
                           bufs=2)
            nc.vector.scalar_tensor_tensor(dt_, psc, WINV, y2ks,
                                           op0=OP.mult, op1=OP.subtract)
            nc.vector.tensor_mul(dt_, dt_, ggb)
            nc.vector.tensor_add(r2[j], y[j], y2ks)
            nc.vector.tensor_add(r2[j], r2[j], dt_)
        tap("r2", r2)
        w1t = ctsb.tile([P, NCH * DFF], bf16, name="w1_t", tag="w1_t")
        nc.sync.dma_start(out=w1t.rearrange("p (i n) -> p i n", n=DFF),
                          in_=w1bd.rearrange("(i p) n -> p i n", p=P))
        w2t = ctsb.tile([P, (DFF // P) * D], bf16, name="w2_t", tag="w2_t")
        nc.sync.dma_start(out=w2t.rearrange("p (i n) -> p i n", n=D),
                          in_=w2bd.rearrange("(i p) n -> p i n", p=P))
        z8 = [rpool.tile([P, 1024], f8, name=f"z8_{i}", tag=f"lnB8_{i}",
                         bufs=1) for i in range(2)]
        ln2 = layernorm(r2, z, 1, out8=z8)
        ln2()
        tap("z", z)

        # --- stage 3: ct cross attention ---
        r3 = mktiles("r3", dt=bf16, tagp="rA")
        ze = mktiles("ze", dt=bf16, tagp="lnA")
        ct_qt = [ctsb.tile([P, 512], bf16, name=f"ct_q{i}", tag=f"ct_q{i}",
                           bufs=1) for i in range(NCH)]
        proj_fm("ct_wq", z8, 512, ct_qt, evict_copy)
        ct_at = [trp.tile([P, 512], bf16, name=f"ct_at{i}", tag=f"at{i}",
                          bufs=2) for i in range(NCH)]
        ct_at8 = mk_at8(trp, "ct_at8")
        fin_ct = attention(ct_qt, ct_kt, ct_vts, ct_at, ct_at8, ctbias,
                           False, "ct")
        fin_ct()

        def evict_resid_r2(j, cs, ce, ps, out_tiles):
            nc.vector.scalar_tensor_tensor(out_tiles[j][:, cs:ce], ps, WINV,
                                           z[j], op0=OP.mult, op1=OP.add)
        tap("ct_at", ct_at)
        proj_fm("ct_wo", ct_at8, 512, r3, evict_resid_r2)
        tap("r3", r3)
        ln3 = layernorm(r3, ze, 2)
        ln3()
        tap("ze", ze)

        # --- stage 4: FFN (bf16 for accuracy) ---
        ffsb = ctx.enter_context(tc.tile_pool(name="ff_sb", bufs=1))
        ht = [ffsb.tile([P, 512], bf16, name=f"ff_h{i}", tag=f"ff_h{i}",
                        bufs=1) for i in range(DFF // P)]
        for jf in range(DFF // P):
            ps = psA.tile([P, 512], mybir.dt.float32, name="ff_ps",
                          tag="pps")
            for i in range(NCH):
                nc.tensor.matmul(ps, w1t[:, i * DFF + jf * P:
                                         i * DFF + (jf + 1) * P],
                                 ze[i], start=(i == 0), stop=(i == NCH - 1))
            if jf % 2 == 0:
                nc.scalar.activation(ht[jf], ps, AF.Relu)
            else:
                nc.vector.tensor_scalar_max(ht[jf], ps, 0.0)
        r4 = mktiles("r4", dt=bf16, tagp="rB")
        for j in range(NCH):
            ps = psA.tile([P, 512], mybir.dt.float32, name="ff_ps2",
                          tag="pps")
            for i in range(DFF // P):
                nc.tensor.matmul(ps, w2t[:, i * D + j * P: i * D + (j + 1) * P],
                                 ht[i], start=(i == 0),
                                 stop=(i == DFF // P - 1))
            nc.vector.tensor_add(r4[j], ps, ze[j])
        fin = [trp.tile([P, 512], bf16, name=f"fin{i}",
                        tag=f"fin{i}", bufs=1) for i in range(NCH)]
        ln4 = layernorm(r4, fin, 3)
        ln4()
        for j in range(NCH):
            nc.sync.dma_start(out=outT[j * P:(j + 1) * P, :], in_=fin[j])

    nc.compile()
    return nc


# ---------------------------------------------------------------------------
# host-side input preparation
# ---------------------------------------------------------------------------

def _prep_shared(inputs):
    """Cast/transform weights shared by every core."""
    sh = {}
    for n in ("sa", "cc", "ct", "ck"):
        for p in ("wq", "wk", "wv", "wo"):
            sh[f"{n}_{p}"] = np.ascontiguousarray(
                (inputs[f"{n}_{p}"].astype(F32) * WSCALE).astype(F8))
    sh["ffn_w1b"] = np.ascontiguousarray(inputs["ffn_w1"].astype(BF16))
    sh["ffn_w2b"] = np.ascontiguousarray(inputs["ffn_w2"].astype(BF16))
    gw = inputs["gate_w"].astype(F32)
    # fold the W_o projections into the gate vectors: the gate logit is
    # computed directly from the attention outputs as
    # u_cc^T cc_at + u_ck^T ck_at with u = W_o @ gate_w
    sh["gwA"] = np.ascontiguousarray(
        (inputs["cc_wo"].astype(F32) @ gw[:D] * GSCALE).astype(F8))
    sh["gwB"] = np.ascontiguousarray(
        (inputs["ck_wo"].astype(F32) @ gw[D:] * GSCALE).astype(F8))
    kl, ql = np.arange(P)[:, None], np.arange(P)[None, :]
    sh["stair"] = np.where(kl <= ql, 0.0, np.float32(-65000.0)).astype(BF16)
    sel8 = np.zeros((8, 4 * P), BF16)
    for hp in range(4):
        sel8[2 * hp, hp * P:hp * P + 64] = 1.0
        sel8[2 * hp + 1, hp * P + 64:(hp + 1) * P] = 1.0
    sh["sel8"] = sel8
    return sh


def _len_bias(L, kts, width=P):
    """[width,1] f32 additive bias for the LAST kv tile."""
    base = (kts - 1) * P
    idx = base + np.arange(width)
    return np.where(idx < L, 0.0, NEG).astype(F32)[:, None]


def _qcols(qh):
    return (np.r_[0:256, 768:1024] if qh == 0 else np.r_[256:768])


def _prep_core(inputs, sh, b, qh, kts_cc, kts_ct):
    KVn = 1024 if qh == 0 else 768
    cols = _qcols(qh)
    m = dict(sh)
    xT = np.ascontiguousarray(inputs["x"][b].T.astype(F32))  # [D, T]
    m["xkvT"] = np.ascontiguousarray(xT[:, :KVn].astype(F8))
    m["xq8T"] = np.ascontiguousarray(xT[:, cols].astype(F8))
    m["xqT"] = np.ascontiguousarray(xT[:, cols].astype(BF16))
    Ls = int(inputs["source_code_len"][b])
    st = np.zeros((D, kts_cc * P), F8)
    st[:, :Ls] = inputs["source_code_enc"][b, :Ls].T.astype(F8)
    m["srcT"] = st
    Lt = int(inputs["template_len"][b])
    tt = np.zeros((D, kts_ct * P), F8)
    tt[:, :Lt] = inputs["template_enc"][b, :Lt].T.astype(F8)
    m["tmplT"] = tt
    m["kwT"] = np.ascontiguousarray(inputs["keywords_enc"][b].T.astype(F8))
    m["cc_bias"] = _len_bias(Ls, kts_cc)
    m["ct_bias"] = _len_bias(Lt, kts_ct)
    m["kw_bias"] = _len_bias(int(inputs["keywords_len"][b]), 1, KW)
    return m


# ---------------------------------------------------------------------------
# concurrent multi-program PJRT runner (adapted from bass2jax.run_bass_via_pjrt)
# ---------------------------------------------------------------------------

def _run_groups(groups):
    """groups: list of (nc, core_ids, in_maps).  Dispatch all groups onto
    their own device subsets, then gather.  Returns {core_id: {name: arr}}."""
    import jax
    import numpy as _np
    from jax.sharding import Mesh, PartitionSpec
    from jax.experimental.shard_map import shard_map
    from concourse import bass2jax
    from concourse.bass2jax import (_bass_exec_p, install_neuronx_cc_hook,
                                    partition_id_tensor)

    install_neuronx_cc_hook()
    devices = jax.devices()

    def make_launch(nc, core_ids, in_maps):
        pname = (nc.partition_id_tensor.name
                 if nc.partition_id_tensor else None)
        in_names, out_names, out_avals, zero_outs = [], [], [], []
        for alloc in nc.m.functions[0].allocations:
            if not isinstance(alloc, mybir.MemoryLocationSet):
                continue
            name = alloc.memorylocations[0].name
            if alloc.kind == "ExternalInput":
                if name == pname:
                    continue
                in_names.append(name)
            elif alloc.kind == "ExternalOutput":
                shape = tuple(alloc.tensor_shape)
                dtype = mybir.dt.np(alloc.dtype)
                out_names.append(name)
                out_avals.append(jax.core.ShapedArray(shape, dtype))
                zero_outs.append(_np.zeros(shape, dtype))
        n_params, n_outs = len(in_names), len(out_avals)
        all_in_names = in_names + out_names
        if pname is not None:
            all_in_names = all_in_names + [pname]

        def _body(*args):
            operands = list(args)
            if pname is not None:
                operands.append(partition_id_tensor())
            outs = _bass_exec_p.bind(
                *operands, out_avals=tuple(out_avals),
                in_names=tuple(all_in_names), out_names=tuple(out_names),
                lowering_input_output_aliases=(),
                sim_require_finite=False, sim_require_nnan=False, nc=nc)
            return tuple(outs)

        donate = tuple(range(n_params, n_params + n_outs))
        devs = [devices[c] for c in core_ids]
        if len(core_ids) == 1:
            fn = jax.jit(_body, donate_argnums=donate, keep_unused=True,
                         device=devs[0])
            args = [in_maps[0][nm] for nm in in_names] + list(zero_outs)
            out_arrs = fn(*args)
            return out_names, out_avals, out_arrs, None
        mesh = Mesh(_np.asarray(devs), ("core",))
        in_specs = (PartitionSpec("core"),) * (n_params + n_outs)
        out_specs = (PartitionSpec("core"),) * n_outs
        fn = jax.jit(shard_map(_body, mesh=mesh, in_specs=in_specs,
                               out_specs=out_specs, check_rep=False),
                     donate_argnums=donate, keep_unused=True)
        cat = [_np.concatenate([_np.asarray(m[nm]) for m in in_maps], axis=0)
               for nm in in_names]
        catz = [_np.zeros((len(core_ids) * z.shape[0], *z.shape[1:]), z.dtype)
                for z in zero_outs]
        out_arrs = fn(*cat, *catz)
        return out_names, out_avals, out_arrs, len(core_ids)

    last_err = None
    for _attempt in range(3):
        try:
            launched = []
            for nc, core_ids, in_maps in groups:
                launched.append((core_ids, make_launch(nc, core_ids, in_maps)))
            results = {}
            for core_ids, (out_names, out_avals, out_arrs, ncores) in launched:
                if ncores is None:
                    results[core_ids[0]] = {nm: _np.asarray(out_arrs[i])
                                            for i, nm in enumerate(out_names)}
                else:
                    for ci, c in enumerate(core_ids):
                        results[c] = {
                            nm: _np.asarray(out_arrs[i]).reshape(
                                ncores, *out_avals[i].shape)[ci]
                            for i, nm in enumerate(out_names)}
            return results
        except Exception as e:  # transient NRT device errors: retry
            last_err = e
            import time as _time
            _time.sleep(2.0)
    raise last_err


_PROGRAM_CACHE = {}
_CACHE_LOCK = threading.Lock()


def _get_program(key):
    with _CACHE_LOCK:
        if key in _PROGRAM_CACHE:
            return _PROGRAM_CACHE[key]
    qh, kts_cc, kts_ct, gate_b, aff = key
    nc = build_program(qh, kts_cc, kts_ct, gate_b=gate_b, apply_affine=aff)
    with _CACHE_LOCK:
        _PROGRAM_CACHE[key] = nc
    return nc


# ---------------------------------------------------------------------------
# entry point
# ---------------------------------------------------------------------------

def kernel(**inputs):
    inputs = {k: np.asarray(v) for k, v in inputs.items()}
    gate_b = float(inputs["gate_b"].reshape(-1)[0])
    aff = not all(
        np.all(inputs[f"ln{j}_g"] == 1.0) and np.all(inputs[f"ln{j}_b"] == 0.0)
        for j in range(1, 5))
    affine_arr = None
    if aff:
        affine_arr = np.zeros((P, NCH * 8), F32)
        for ln in range(4):
            g = inputs[f"ln{ln + 1}_g"].astype(F32).reshape(NCH, P).T
            bb = inputs[f"ln{ln + 1}_b"].astype(F32).reshape(NCH, P).T
            affine_arr[:, ln * 2 * NCH: ln * 2 * NCH + NCH] = g
            affine_arr[:, ln * 2 * NCH + NCH: (ln + 1) * 2 * NCH] = bb

    sh = _prep_shared(inputs)
    # core -> (program key, in_map)
    core_keys, core_maps = [], []
    for c in range(8):
        b, qh = c // 2, c % 2
        kts_cc = max(1, -(-int(inputs["source_code_len"][b]) // P))
        kts_ct = max(1, -(-int(inputs["template_len"][b]) // P))
        key = (qh, kts_cc, kts_ct, gate_b, aff)
        m = _prep_core(inputs, sh, b, qh, kts_cc, kts_ct)
        if aff:
            m["ln_affine"] = affine_arr
        core_keys.append(key)
        core_maps.append(m)

    # build distinct programs (parallel threads: walrus compile is subprocess)
    distinct = sorted(set(core_keys))
    threads = [threading.Thread(target=_get_program, args=(k,))
               for k in distinct]
    for t in threads:
        t.start()
    for t in threads:
        t.join()

    groups = []
    for key in distinct:
        cores = [c for c in range(8) if core_keys[c] == key]
        groups.append((_get_program(key), cores, [core_maps[c] for c in cores]))

    results = _run_groups(groups)

    out = np.empty((B, T, D), np.float32)
    for c in range(8):
        b, qh = c // 2, c % 2
        out[b, _qcols(qh), :] = results[c]["outT"].T.astype(np.float32)
    return out

